# revision 24
# baseline (speedup 1.0000x reference)
"""2-layer GIN + attentional pooling on 8 Trainium2 NeuronCores (Bass/Tile).

v2 rewrite of the gather/cumsum baseline:
  - Nodes split into 8 graph-aligned ownership ranges (one per core); each
    core processes edges whose dst it owns, bucketed by 32768-node src block
    (one per GPSIMD core-group) and dst-ordered within 1024-node chunks.
  - Segment sums over dst: DVE prefix-scan over the dst-sorted edge stream,
    ap_gather of the cumsum at host-known segment ends, shifted subtraction.
  - GIN linear fused into the block fold: (x + A.x) @ w == w_blk-fold(P) + w.x,
    so each 512-node tile is 2 (L1) / 3 (L2) fp16 matmuls, single PE pass.
  - h1 exchanged with AllGather (f16, feature pairs (j, j+16) interleaved for
    the 4-byte d=2 gather granule); a plain [32, N] copy kept for the local
    self term.
  - Pooling: exact per-graph softmax. Nodes re-laid 4 graph-aligned quarters
    x 32 features across 128 partitions; gate/attn MLPs as block-diagonal
    128x128 fp16 matmuls; per-graph gate max via ap_gather into padded
    per-graph slots + 3D tensor_reduce; exp(gate - max) with the max
    broadcast back by a second tiny gather; denominator and weighted sums via
    per-quarter prefix scans probed at graph ends.
"""
import os
import sys

os.environ.setdefault("NEURON_RT_RESET_CORES", "1")
sys.path.insert(0, '/opt/trn_rl_repo')

import numpy as np


# -- NTFF profiling hook shim (optional; enables trace=True under axon) ----
def _install_ntff_shim():
    import types
    try:
        import antenv
        if 'antenv.axon_hooks' in sys.modules:
            return
        hooks = types.ModuleType('antenv.axon_hooks')
        _state = {'hook': None}
        hooks.set_axon_ntff_profile_hook = lambda h: _state.__setitem__('hook', h)
        hooks.get_axon_ntff_profile_hook = lambda: _state['hook']
        sys.modules['antenv.axon_hooks'] = hooks
        antenv.axon_hooks = hooks
        from trn_agent_boot.trn_boot import _ntff_profile_via_ctypes
        h = _ntff_profile_via_ctypes('/opt/axon/libaxon_pjrt.so')
        if h is not None:
            hooks.set_axon_ntff_profile_hook(h)
    except Exception:
        pass


_install_ntff_shim()

N_NODES = 262144
N_GRAPHS = 1024
C_IN = 16
H = 32
NC = 8
BLK = 32768
NCH = 1024                 # dst nodes per chunk
NCHUNK = 33
ECH = 2368                 # edge capacity per (block, chunk); max seen 2220
NMAX = NCH * NCHUNK        # 33792
QSTEP = NMAX // 4          # 8448: fixed quarter stride (same on all cores)
QW = 9728                  # 19*512: fixed quarter window width
QOFF = (0, QSTEP - 512, 2 * QSTEP - 512, 3 * QSTEP - 512)  # static offsets
H2PAD = QOFF[3] + QW - NMAX  # 768 zero-padded columns after h2
NSP = 24                   # graph slots per gmax gather pass
NS = 2 * NSP               # graph slots per quarter window (max seen 34)
SW = 320                   # slot width >= max graph size (max seen 317)
SENT = -1.0e5
TILE_N = 512
MAX_WAITS = 1

_cache = {}


def _split_multi_waits(nc, mybir, max_waits=MAX_WAITS):
    n_split = 0
    for fn in nc.m.functions:
        for bb in fn.blocks:
            out = []
            for ins in bb.instructions:
                si = ins.sync_info
                if si is not None and si.on_wait and len(si.on_wait) > max_waits:
                    waits = list(si.on_wait)
                    extra = waits[:-max_waits]
                    keep = waits[-max_waits:]
                    for i in range(0, len(extra), max_waits):
                        group = extra[i:i + max_waits]
                        nop = mybir.InstNoOp(
                            name=f"waitsplit_{nc.next_id()}",
                            sync_info=mybir.SyncInfo(on_wait=group, on_update=[]),
                            bass_nofuse=True,
                            engine=ins.engine,
                        )
                        out.append(nop)
                        n_split += 1
                    si.on_wait = keep
                out.append(ins)
            bb.instructions = out
    return n_split


def _wrap_idx(vals, group, arr, col0=0):
    """Wrapped ap_gather index layout: value i -> arr[16g + i%16, col0 + i//16]."""
    n = len(vals)
    assert n % 16 == 0
    v = np.asarray(vals, dtype=np.int16).reshape(n // 16, 16).T
    arr[16 * group:16 * group + 16, col0:col0 + n // 16] = v


def _register_cumsum():
    from concourse import dve_ops
    from concourse.dve_spec import Spec, Src0, C0, AluOp, lower
    import concourse.dve_spec as ds
    from concourse.dve_uop import DveOpSpec
    for op in dve_ops.OPS:
        if op.name == "CUMSUM_ANT":
            return op
    spec = Spec(
        body=ds.scan(AluOp.ADD, Src0, init=C0),
        reference=lambda in0, s0: np.cumsum(in0.astype(np.float32), axis=-1) + s0,
    )
    shas = {}
    for ver in ("v3", "v4"):
        uops = lower(spec, ver=ver)
        shas[ver] = DveOpSpec(name="CUMSUM_ANT", opcode=1, uops=uops,
                              rd1_en=False).sha(ver)
    op = dve_ops.DveOp("CUMSUM_ANT", spec, subdim=False, uops_sha=shas)
    dve_ops.OPS.append(op)
    dve_ops.CUSTOM_DVE_SPECS["CUMSUM_ANT"] = spec
    dve_ops._SUB_OPCODE_FOR_NAME["CUMSUM_ANT"] = \
        max(dve_ops._SUB_OPCODE_FOR_NAME.values()) + 1
    return op


# ================================================================ host prep
def _prep(edge_index, batch_vec):
    src = np.asarray(edge_index[0], dtype=np.int64)
    dst = np.asarray(edge_index[1], dtype=np.int64)
    bv = np.asarray(batch_vec, dtype=np.int64)

    gstart = np.searchsorted(bv, np.arange(N_GRAPHS))
    bounds = [0]
    for c in range(1, NC):
        target = c * (N_NODES // NC)
        gi = np.searchsorted(gstart, target)
        cand = []
        if gi < N_GRAPHS:
            cand.append(int(gstart[gi]))
        if gi > 0:
            cand.append(int(gstart[gi - 1]))
        bounds.append(min(cand, key=lambda v: abs(v - target)))
    bounds.append(N_NODES)
    n_lo = np.array(bounds[:-1])
    n_hi = np.array(bounds[1:])
    sizes = n_hi - n_lo
    assert sizes.max() <= NMAX, sizes
    g_lo = np.searchsorted(gstart, n_lo)
    g_hi = np.searchsorted(gstart, n_hi)

    owner = np.searchsorted(n_hi, dst, side='right')

    cores = []
    for c in range(NC):
        m = owner == c
        csrc = src[m]
        cdst_local = dst[m] - n_lo[c]
        size_c = int(sizes[c])

        ge = np.zeros((128, NCHUNK * ECH // 16), np.int16)
        gd = np.zeros((128, NCHUNK * NCH // 16), np.int16)

        blk_of = csrc >> 15
        src_local_all = (csrc & (BLK - 1))

        for k in range(NC):
            bm = blk_of == k
            bsrc = src_local_all[bm]
            bdst = cdst_local[bm]
            order = np.argsort(bdst, kind='stable')
            bsrc = bsrc[order].astype(np.int16)
            bdst = bdst[order]
            cnt = np.bincount(bdst, minlength=NMAX)
            cum = np.concatenate([[0], np.cumsum(cnt)])

            for ch in range(NCHUNK):
                a, b = ch * NCH, (ch + 1) * NCH
                e0, e1 = cum[a], cum[b]
                ne = int(e1 - e0)
                assert ne <= ECH, (c, k, ch, ne, ECH)
                ev = np.zeros(ECH, np.int16)
                ev[:ne] = bsrc[e0:e1]
                _wrap_idx(ev, k, ge, col0=ch * ECH // 16)
                ends = (cum[a + 1:b + 1] - e0).astype(np.int16)
                _wrap_idx(ends, k, gd, col0=ch * NCH // 16)

        # ---- pooling quarter windows (fixed offsets, per-core content) ----
        glo, ghi = int(g_lo[c]), int(g_hi[c])
        ls = (gstart[glo:ghi] - n_lo[c]).astype(np.int64)       # graph starts
        le = np.concatenate([ls[1:], [size_c]]).astype(np.int64)  # graph ends
        q_of = ls // QSTEP  # quarter window owning each graph (by start node)

        gsa = np.zeros((128, NSP * SW // 16), np.int16)
        gsb = np.zeros((128, NSP * SW // 16), np.int16)
        gnode = np.zeros((128, QW // 16), np.int16)
        gend = np.zeros((128, NS // 16), np.int16)
        slot_map = np.full((4, NS), -1, np.int64)
        for q in range(4):
            off = QOFF[q]
            sel = np.where(q_of == q)[0]
            ngq = len(sel)
            assert ngq <= NS, (c, q, ngq)
            starts = ls[sel] - off
            ends = le[sel] - off
            lens = ends - starts
            if ngq:
                assert starts.min() >= 0 and ends.max() <= QW, (c, q)
                assert lens.max() <= SW, (c, q, lens.max())
            for s in range(ngq):
                slot_map[q, s] = int(sel[s])  # graph idx local to core

            # slot gather streams (two passes of NSP slots each)
            for p, gs_arr in ((0, gsa), (1, gsb)):
                ev = np.full(NSP * SW, QW, np.int16)  # sentinel column
                for si in range(NSP):
                    s = p * NSP + si
                    if s < ngq:
                        w = np.minimum(np.arange(SW), lens[s] - 1)
                        ev[si * SW:(si + 1) * SW] = (starts[s] + w).astype(np.int16)
                _wrap_idx(ev, 2 * q, gs_arr)
                _wrap_idx(ev, 2 * q + 1, gs_arr)

            # node -> slot stream; foreign/pad columns -> slot NS (+1e5 max)
            evn = np.full(QW, NS, np.int16)
            for s in range(ngq):
                evn[starts[s]:ends[s]] = s
            _wrap_idx(evn, 2 * q, gnode)
            _wrap_idx(evn, 2 * q + 1, gnode)

            # slot -> cumsum end-probe offset stream
            eve = np.zeros(NS, np.int16)
            prev = int(starts[0]) if ngq else 0
            for s in range(NS):
                if s < ngq:
                    prev = int(ends[s])
                eve[s] = prev
            _wrap_idx(eve, 2 * q, gend)
            _wrap_idx(eve, 2 * q + 1, gend)

        cores.append(dict(
            n_lo=int(n_lo[c]), size=size_c, g_lo=glo, g_hi=ghi,
            ge=ge, gd=gd, gsa=gsa, gsb=gsb, gnode=gnode, gend=gend,
            slot_map=slot_map,
        ))
    return cores, [int(b) for b in bounds]


# ================================================================ device
def _build_program(bounds):
    from concourse import bacc, tile
    from concourse.bass import mybir

    CUMSUM = _register_cumsum()

    f32 = mybir.dt.float32
    f16 = mybir.dt.float16
    i16 = mybir.dt.int16
    RELU = mybir.ActivationFunctionType.Relu
    EXP = mybir.ActivationFunctionType.Exp
    IDENT = mybir.ActivationFunctionType.Identity
    SUB = mybir.AluOpType.subtract
    MUL = mybir.AluOpType.mult

    nc = bacc.Bacc("TRN2", target_bir_lowering=False, debug=False, num_devices=NC)

    def din(name, shape, dt):
        return nc.dram_tensor(name, shape, dt, kind="ExternalInput")

    xt_in = din("xt", [128, BLK], f32)
    xo_in = din("xo", [16, NMAX], f16)
    ge_in = din("ge", [128, NCHUNK * ECH // 16], i16)
    gd_in = din("gd", [128, NCHUNK * NCH // 16], i16)
    gsa_in = din("gsa", [128, NSP * SW // 16], i16)
    gsb_in = din("gsb", [128, NSP * SW // 16], i16)
    gnode_in = din("gnode", [128, QW // 16], i16)
    gend_in = din("gend", [128, NS // 16], i16)
    wp16_in = din("wp16", [128, 1216], f16)
    wp32_in = din("wp32", [128, 16], f32)

    out_g = nc.dram_tensor("outg", [4, NS], f32, kind="ExternalOutput")

    h1i_own = nc.dram_tensor("h1i_own", [16, NMAX, 2], f16)
    h1i_all = nc.dram_tensor("h1i_all", [NC * 16, NMAX, 2], f16,
                             addr_space="Shared")
    h2_dram = nc.dram_tensor("h2d", [32, NMAX + H2PAD], f16)

    with tile.TileContext(nc) as tc:
        with (
            tc.tile_pool(name="cw", bufs=1) as cw,
        ):
            wp16 = cw.tile([128, 1216], f16, name="wp16")
            nc.sync.dma_start(wp16[:], wp16_in.ap()[:])
            wp32 = cw.tile([128, 16], f32, name="wp32")
            nc.sync.dma_start(wp32[:], wp32_in.ap()[:])

            zpad = cw.tile([32, H2PAD], f16, name="zpad")
            nc.vector.memset(zpad[:], 0.0)
            nc.sync.dma_start(h2_dram.ap()[:, NMAX:NMAX + H2PAD], zpad[:])

            w1blk_a = wp16[:, 0:16]
            w1blk_b = wp16[:, 16:32]
            w1s_a = wp16[0:16, 32:48]
            w1s_b = wp16[0:16, 48:64]
            w2blk0 = wp16[:, 64:96]
            w2blk1 = wp16[:, 96:128]
            w2s_a = wp16[0:16, 128:160]
            w2s_b = wp16[0:16, 160:192]
            BD = {}
            for i, nm in enumerate(("gw1", "gw2", "gw3r", "aw1", "aw2",
                                    "fw1", "fw2", "fw3r")):
                BD[nm] = wp16[:, 192 + 128 * i:192 + 128 * (i + 1)]
            b1a = wp32[0:16, 0:1]
            b1b = wp32[0:16, 1:2]
            b2f = wp32[0:32, 2:3]
            gb1t = wp32[:, 3:4]
            gb2t = wp32[:, 4:5]
            gb3t = wp32[:, 5:6]
            ab1t = wp32[:, 6:7]
            ab2t = wp32[:, 7:8]
            fb1t = wp32[:, 8:9]
            fb2t = wp32[:, 9:10]
            fb3t = wp32[:, 10:11]

            with (
                tc.tile_pool(name="tbl", bufs=1) as tblp,
            ):
                # ---------------- Layer 1 ----------------
                table1 = tblp.tile([128, BLK], f32, tag="table")
                nc.sync.dma_start(table1[:], xt_in.ap()[:])

                with nc.named_scope("L1"), (
                    tc.tile_pool(name="ip", bufs=4)) as ip, (
                    tc.tile_pool(name="xp", bufs=3)) as xp, (
                    tc.tile_pool(name="cp", bufs=2)) as cp, (
                    tc.tile_pool(name="gp", bufs=2)) as gp, (
                    tc.tile_pool(name="op", bufs=3)) as op, (
                    tc.tile_pool(name="pp", bufs=3, space="PSUM")) as pp:
                    for ch in range(NCHUNK):
                        gidx = ip.tile([128, ECH // 16], i16, tag="gidx")
                        nc.sync.dma_start(
                            gidx[:],
                            ge_in.ap()[:, ch * ECH // 16:(ch + 1) * ECH // 16])
                        didx = ip.tile([128, NCH // 16], i16, tag="didx")
                        nc.sync.dma_start(
                            didx[:],
                            gd_in.ap()[:, ch * NCH // 16:(ch + 1) * NCH // 16])
                        xoc = xp.tile([16, NCH], f16, tag="xoc")
                        nc.sync.dma_start(
                            xoc[:], xo_in.ap()[:, ch * NCH:(ch + 1) * NCH])

                        cs = cp.tile([128, 1 + ECH], f32, tag="cs")
                        nc.vector.memset(cs[:, 0:1], 0.0)
                        nc.gpsimd.ap_gather(
                            cs[:, 1:], table1[:], gidx[:],
                            channels=128, num_elems=BLK, d=1, num_idxs=ECH)
                        nc.vector._custom_dve(
                            CUMSUM, out=cs[:, 1:], in0=cs[:, 1:], s0=0.0)

                        G = gp.tile([128, 1 + NCH], f32, tag="G")
                        nc.vector.memset(G[:, 0:1], 0.0)
                        nc.gpsimd.ap_gather(
                            G[:, 1:], cs[:], didx[:],
                            channels=128, num_elems=1 + ECH, d=1, num_idxs=NCH)
                        P = gp.tile([128, NCH], f16, tag="P")
                        nc.vector.tensor_tensor(P[:], G[:, 1:], G[:, :-1], SUB)

                        for t0 in range(0, NCH, TILE_N):
                            tn = TILE_N
                            sl = slice(t0, t0 + tn)
                            pha = pp.tile([16, tn], f32, tag="psa")
                            nc.tensor.matmul(pha[:], w1blk_a, P[:, sl],
                                             start=True, stop=False)
                            nc.tensor.matmul(pha[:], w1s_a, xoc[:, sl],
                                             start=False, stop=True)
                            phb = pp.tile([16, tn], f32, tag="psb")
                            nc.tensor.matmul(phb[:], w1blk_b, P[:, sl],
                                             start=True, stop=False)
                            nc.tensor.matmul(phb[:], w1s_b, xoc[:, sl],
                                             start=False, stop=True)
                            he3 = op.tile([16, tn, 2], f16, tag="he3")
                            nc.scalar.activation(he3[:, :, 0], pha[:], RELU,
                                                 bias=b1a)
                            nc.scalar.activation(he3[:, :, 1], phb[:], RELU,
                                                 bias=b1b)
                            col = ch * NCH + t0
                            nc.sync.dma_start(
                                h1i_own.ap()[:, col:col + tn, :], he3[:])

                # ---------------- exchange ----------------
                with nc.named_scope("AG"):
                    nc.gpsimd.collective_compute(
                        "AllGather", mybir.AluOpType.bypass,
                        replica_groups=[list(range(NC))],
                        ins=[h1i_own.ap()[:]],
                        outs=[h1i_all.ap()[:]],
                    )

                # ---------------- table2 ----------------
                table2 = tblp.tile([128, BLK, 2], f16, tag="table")
                with nc.named_scope("T2"):
                    for k in range(NC):
                        lo, hi = k * BLK, (k + 1) * BLK
                        pos = lo
                        while pos < hi:
                            c2 = next(i for i in range(NC)
                                      if bounds[i] <= pos < bounds[i + 1])
                            seg_end = min(hi, bounds[c2 + 1])
                            ln = seg_end - pos
                            local = pos - bounds[c2]
                            nc.sync.dma_start(
                                table2[16 * k:16 * (k + 1),
                                       pos - lo:pos - lo + ln, :],
                                h1i_all.ap()[16 * c2:16 * (c2 + 1),
                                             local:local + ln, :])
                            pos = seg_end

                # ---------------- Layer 2 ----------------
                with nc.named_scope("L2"), (
                    tc.tile_pool(name="ip2", bufs=4)) as ip, (
                    tc.tile_pool(name="xp2", bufs=3)) as xp, (
                    tc.tile_pool(name="cp2", bufs=2)) as cp, (
                    tc.tile_pool(name="gp2", bufs=2)) as gp, (
                    tc.tile_pool(name="op2", bufs=3)) as op, (
                    tc.tile_pool(name="pp2", bufs=4, space="PSUM")) as pp:
                    for ch in range(NCHUNK):
                        gidx = ip.tile([128, ECH // 16], i16, tag="gidx")
                        nc.sync.dma_start(
                            gidx[:],
                            ge_in.ap()[:, ch * ECH // 16:(ch + 1) * ECH // 16])
                        didx = ip.tile([128, NCH // 16], i16, tag="didx")
                        nc.sync.dma_start(
                            didx[:],
                            gd_in.ap()[:, ch * NCH // 16:(ch + 1) * NCH // 16])
                        h1c = xp.tile([16, NCH, 2], f16, tag="h1c")
                        nc.sync.dma_start(
                            h1c[:], h1i_own.ap()[:, ch * NCH:(ch + 1) * NCH, :])

                        stage = cp.tile([128, ECH, 2], f16, tag="stage")
                        nc.gpsimd.ap_gather(
                            stage[:], table2[:], gidx[:],
                            channels=128, num_elems=BLK, d=2, num_idxs=ECH)
                        cs2 = cp.tile([128, 1 + ECH, 2], f32, tag="cs2", bufs=1)
                        nc.vector.memset(cs2[:, 0:1, :], 0.0)
                        nc.vector._custom_dve(
                            CUMSUM, out=cs2[:, 1:, 0], in0=stage[:, :, 0], s0=0.0)
                        nc.vector._custom_dve(
                            CUMSUM, out=cs2[:, 1:, 1], in0=stage[:, :, 1], s0=0.0)

                        G2 = gp.tile([128, 1 + NCH, 2], f32, tag="G2", bufs=1)
                        nc.vector.memset(G2[:, 0:1, :], 0.0)
                        nc.gpsimd.ap_gather(
                            G2[:, 1:, :], cs2[:], didx[:],
                            channels=128, num_elems=1 + ECH, d=2, num_idxs=NCH)
                        P2 = gp.tile([128, NCH, 2], f16, tag="P2")
                        nc.vector.tensor_tensor(P2[:], G2[:, 1:, :], G2[:, :-1, :],
                                                SUB)

                        for t0 in range(0, NCH, TILE_N):
                            tn = TILE_N
                            sl = slice(t0, t0 + tn)
                            ph2 = pp.tile([H, tn], f32, tag="ps2")
                            nc.tensor.matmul(ph2[:], w2blk0, P2[:, sl, 0],
                                             start=True, stop=False)
                            nc.tensor.matmul(ph2[:], w2blk1, P2[:, sl, 1],
                                             start=False, stop=False)
                            nc.tensor.matmul(ph2[:], w2s_a, h1c[:, sl, 0],
                                             start=False, stop=False)
                            nc.tensor.matmul(ph2[:], w2s_b, h1c[:, sl, 1],
                                             start=False, stop=True)
                            h2t = op.tile([32, tn], f16, tag="h2t")
                            nc.scalar.activation(h2t[:], ph2[:], RELU, bias=b2f)
                            col = ch * NCH + t0
                            nc.sync.dma_start(
                                h2_dram.ap()[:, col:col + tn], h2t[:])

            # ---------------- pooling ----------------
            with (
                tc.tile_pool(name="p3", bufs=1) as p3,
                tc.tile_pool(name="tp", bufs=2) as tp,
                tc.tile_pool(name="pq", bufs=4, space="PSUM") as pq,
            ):
                gsa = p3.tile([128, NSP * SW // 16], i16)
                nc.sync.dma_start(gsa[:], gsa_in.ap()[:])
                gsb = p3.tile([128, NSP * SW // 16], i16)
                nc.sync.dma_start(gsb[:], gsb_in.ap()[:])
                gnode = p3.tile([128, QW // 16], i16)
                nc.sync.dma_start(gnode[:], gnode_in.ap()[:])
                gend = p3.tile([128, NS // 16], i16)
                nc.sync.dma_start(gend[:], gend_in.ap()[:])

                h2q = p3.tile([128, QW], f16)
                for q in range(4):
                    nc.sync.dma_start(
                        h2q[32 * q:32 * (q + 1), :],
                        h2_dram.ap()[:, QOFF[q]:QOFF[q] + QW])

                gate_q = p3.tile([128, QW + 1], f32)
                with nc.named_scope("PMLP"):
                    for t0 in range(0, QW, TILE_N):
                        sl = slice(t0, t0 + TILE_N)
                        pg1 = pq.tile([128, TILE_N], f32, tag="psq")
                        nc.tensor.matmul(pg1[:], BD["gw1"], h2q[:, sl],
                                         start=True, stop=True)
                        g1s = tp.tile([128, TILE_N], f16, tag="g1s")
                        nc.scalar.activation(g1s[:], pg1[:], RELU, bias=gb1t)
                        pg2 = pq.tile([128, TILE_N], f32, tag="psq")
                        nc.tensor.matmul(pg2[:], BD["gw2"], g1s[:],
                                         start=True, stop=True)
                        g2s = tp.tile([128, TILE_N], f16, tag="g2s")
                        nc.scalar.activation(g2s[:], pg2[:], RELU, bias=gb2t)
                        pg3 = pq.tile([128, TILE_N], f32, tag="psq")
                        nc.tensor.matmul(pg3[:], BD["gw3r"], g2s[:],
                                         start=True, stop=True)
                        nc.scalar.activation(gate_q[:, sl], pg3[:], IDENT,
                                             bias=gb3t)

                # sentinel column for empty-slot gathers
                nc.vector.memset(gate_q[:, QW:QW + 1], SENT)

                # per-graph max; extra slot NS = +1e5 kills foreign columns
                gmax = p3.tile([128, NS + 1], f32)
                nc.vector.memset(gmax[:, NS:NS + 1], 1.0e5)
                NSPH = NSP // 2
                HC = NSPH * SW // 16  # idx cols per half-pass
                with nc.named_scope("GMAX"):
                    for p in range(4):
                        gs_t = (gsa, gsa, gsb, gsb)[p]
                        hsl = slice((p % 2) * HC, (p % 2 + 1) * HC)
                        Zs = tp.tile([128, NSPH, SW], f32, tag="Zs", bufs=1)
                        nc.gpsimd.ap_gather(
                            Zs[:], gate_q[:], gs_t[:, hsl],
                            channels=128, num_elems=QW + 1, d=1,
                            num_idxs=NSPH * SW)
                        nc.vector.tensor_reduce(
                            gmax[:, p * NSPH:(p + 1) * NSPH], Zs[:],
                            mybir.AxisListType.X, mybir.AluOpType.max)

                # second pass: attn mlp, exp, prefix sums
                csE = p3.tile([128, 1 + QW], f32)
                csW = p3.tile([128, 1 + QW], f32)
                nc.vector.memset(csE[:, 0:1], 0.0)
                nc.vector.memset(csW[:, 0:1], 0.0)
                with nc.named_scope("PATT"):
                    for t0 in range(0, QW, TILE_N):
                        sl = slice(t0, t0 + TILE_N)
                        Mt = tp.tile([128, TILE_N], f32, tag="Mt")
                        nc.gpsimd.ap_gather(
                            Mt[:], gmax[:], gnode[:, t0 // 16:(t0 + TILE_N) // 16],
                            channels=128, num_elems=NS + 1, d=1, num_idxs=TILE_N)
                        pt1 = pq.tile([128, TILE_N], f32, tag="psq")
                        nc.tensor.matmul(pt1[:], BD["aw1"], h2q[:, sl],
                                         start=True, stop=True)
                        t1s = tp.tile([128, TILE_N], f16, tag="g1s")
                        nc.scalar.activation(t1s[:], pt1[:], RELU, bias=ab1t)
                        pt2 = pq.tile([128, TILE_N], f32, tag="psq")
                        nc.tensor.matmul(pt2[:], BD["aw2"], t1s[:],
                                         start=True, stop=True)
                        t2s = tp.tile([128, TILE_N], f32, tag="t2s")
                        nc.scalar.activation(t2s[:], pt2[:], RELU, bias=ab2t)

                        Ep = tp.tile([128, TILE_N], f32, tag="Ep")
                        nc.vector.tensor_tensor(Ep[:], gate_q[:, sl], Mt[:], SUB)
                        Ee = tp.tile([128, TILE_N], f32, tag="Ee")
                        nc.scalar.activation(Ee[:], Ep[:], EXP)
                        wt = tp.tile([128, TILE_N], f32, tag="wt")
                        nc.vector.tensor_tensor(wt[:], Ee[:], t2s[:], MUL)
                        nc.vector._custom_dve(
                            CUMSUM, out=csE[:, 1 + t0:1 + t0 + TILE_N],
                            in0=Ee[:], s0=csE[:, t0:t0 + 1])
                        nc.vector._custom_dve(
                            CUMSUM, out=csW[:, 1 + t0:1 + t0 + TILE_N],
                            in0=wt[:], s0=csW[:, t0:t0 + 1])

                with nc.named_scope("PFIN"):
                    GdE = p3.tile([128, 1 + NS], f32)
                    GdW = p3.tile([128, 1 + NS], f32)
                    nc.vector.memset(GdE[:, 0:1], 0.0)
                    nc.vector.memset(GdW[:, 0:1], 0.0)
                    nc.gpsimd.ap_gather(
                        GdE[:, 1:], csE[:], gend[:],
                        channels=128, num_elems=1 + QW, d=1, num_idxs=NS)
                    nc.gpsimd.ap_gather(
                        GdW[:, 1:], csW[:], gend[:],
                        channels=128, num_elems=1 + QW, d=1, num_idxs=NS)
                    denom = p3.tile([128, NS], f32)
                    num = p3.tile([128, NS], f32)
                    nc.vector.tensor_tensor(denom[:], GdE[:, 1:], GdE[:, :-1],
                                            SUB)
                    nc.vector.tensor_tensor(num[:], GdW[:, 1:], GdW[:, :-1], SUB)
                    nc.vector.tensor_scalar_max(denom[:], denom[:], 1e-16)
                    rec = p3.tile([128, NS], f32)
                    nc.vector.reciprocal(rec[:], denom[:])
                    pooled = p3.tile([128, NS], f16)
                    nc.vector.tensor_tensor(pooled[:], num[:], rec[:], MUL)

                    pc1 = pq.tile([128, NS], f32, tag="psc", bufs=2)
                    nc.tensor.matmul(pc1[:], BD["fw1"], pooled[:],
                                     start=True, stop=True)
                    c1s = p3.tile([128, NS], f16)
                    nc.scalar.activation(c1s[:], pc1[:], RELU, bias=fb1t)
                    pc2 = pq.tile([128, NS], f32, tag="psc", bufs=2)
                    nc.tensor.matmul(pc2[:], BD["fw2"], c1s[:],
                                     start=True, stop=True)
                    c2s = p3.tile([128, NS], f16)
                    nc.scalar.activation(c2s[:], pc2[:], RELU, bias=fb2t)
                    pc3 = pq.tile([128, NS], f32, tag="psc", bufs=2)
                    nc.tensor.matmul(pc3[:], BD["fw3r"], c2s[:],
                                     start=True, stop=True)
                    o3 = p3.tile([128, NS], f32)
                    nc.scalar.activation(o3[:], pc3[:], IDENT, bias=fb3t)
                    for q in range(4):
                        nc.sync.dma_start(out_g.ap()[q:q + 1, :],
                                          o3[32 * q:32 * q + 1, :])

    nc.compile()
    _split_multi_waits(nc, mybir)
    return nc


# ================================================================ entry
def kernel(x, w1, b1, w2, b2, gw1, gb1, gw2, gb2, gw3, gb3,
           aw1, ab1, aw2, ab2, fw1, fb1, fw2, fb2, fw3, fb3,
           edge_index, batch_vec, num_graphs):
    from concourse.bass_utils import run_bass_kernel_spmd

    x = np.asarray(x, np.float32)
    cores, bounds = _prep(edge_index, batch_vec)

    f32a = lambda a: np.asarray(a, np.float32)
    f16a = lambda a: np.ascontiguousarray(np.asarray(a, np.float32)
                                          .astype(np.float16))

    w1n, w2n = f32a(w1), f32a(w2)

    xt = np.zeros((128, BLK), np.float32)
    for k in range(NC):
        xt[16 * k:16 * (k + 1), :] = x[BLK * k:BLK * (k + 1), :].T

    def bd4(w):
        out = np.zeros((128, 128), np.float32)
        for q in range(4):
            out[32 * q:32 * (q + 1), 32 * q:32 * (q + 1)] = w
        return out

    gw3r = np.tile(f32a(gw3).reshape(H, 1), (1, H))
    fw3r = np.tile(f32a(fw3).reshape(H, 1), (1, H))

    wp16 = np.zeros((128, 1216), np.float32)
    wp16[:, 0:16] = np.tile(w1n[:, 0:16], (8, 1))
    wp16[:, 16:32] = np.tile(w1n[:, 16:32], (8, 1))
    wp16[0:16, 32:48] = w1n[:, 0:16]
    wp16[0:16, 48:64] = w1n[:, 16:32]
    wp16[:, 64:96] = np.tile(w2n[0:16, :], (8, 1))
    wp16[:, 96:128] = np.tile(w2n[16:32, :], (8, 1))
    wp16[0:16, 128:160] = w2n[0:16, :]
    wp16[0:16, 160:192] = w2n[16:32, :]
    for i, w in enumerate((gw1, gw2, gw3r, aw1, aw2, fw1, fw2, fw3r)):
        wp16[:, 192 + 128 * i:192 + 128 * (i + 1)] = bd4(f32a(w))
    wp16 = wp16.astype(np.float16)

    def t4(b):
        return np.tile(f32a(b).reshape(H), 4)

    wp32 = np.zeros((128, 16), np.float32)
    wp32[0:16, 0] = f32a(b1).reshape(-1)[0:16]
    wp32[0:16, 1] = f32a(b1).reshape(-1)[16:32]
    wp32[0:32, 2] = f32a(b2).reshape(-1)
    wp32[:, 3] = t4(gb1)
    wp32[:, 4] = t4(gb2)
    wp32[:, 5] = float(np.asarray(gb3).reshape(-1)[0])
    wp32[:, 6] = t4(ab1)
    wp32[:, 7] = t4(ab2)
    wp32[:, 8] = t4(fb1)
    wp32[:, 9] = t4(fb2)
    wp32[:, 10] = float(np.asarray(fb3).reshape(-1)[0])

    common = dict(xt=xt, wp16=wp16, wp32=wp32)

    in_maps = []
    for c, info in enumerate(cores):
        xo = np.zeros((16, NMAX), np.float16)
        xo[:, :info['size']] = \
            x[info['n_lo']:info['n_lo'] + info['size'], :].T.astype(np.float16)
        m = dict(common)
        m.update(xo=xo, ge=info['ge'], gd=info['gd'], gsa=info['gsa'],
                 gsb=info['gsb'], gnode=info['gnode'], gend=info['gend'])
        in_maps.append(m)

    key = tuple(bounds)
    if _cache.get('key') != key:
        _cache['nc'] = _build_program(bounds)
        _cache['key'] = key
    ncp = _cache['nc']

    res = run_bass_kernel_spmd(ncp, in_maps, core_ids=list(range(NC)),
                               trace=bool(os.environ.get("KERNEL_TRACE")))
    _cache['last_results'] = res

    out = np.zeros((N_GRAPHS, 1), np.float32)
    for c, info in enumerate(cores):
        vals = np.asarray(res.results[c]["outg"])  # [4, NS]
        for q in range(4):
            for s in range(NS):
                g = info['slot_map'][q, s]
                if g >= 0:
                    out[info['g_lo'] + g, 0] = vals[q, s]
    return out


# revision 29
# speedup vs baseline: 1.1869x; 1.1869x over previous
"""2-layer GIN + attentional pooling on 8 Trainium2 NeuronCores (Bass/Tile).

v2 rewrite of the gather/cumsum baseline:
  - Nodes split into 8 graph-aligned ownership ranges (one per core); each
    core processes edges whose dst it owns, bucketed by 32768-node src block
    (one per GPSIMD core-group) and dst-ordered within 1024-node chunks.
  - Segment sums over dst: DVE prefix-scan over the dst-sorted edge stream,
    ap_gather of the cumsum at host-known segment ends, shifted subtraction.
  - GIN linear fused into the block fold: (x + A.x) @ w == w_blk-fold(P) + w.x,
    so each 512-node tile is 2 (L1) / 3 (L2) fp16 matmuls, single PE pass.
  - h1 exchanged with AllGather (f16, feature pairs (j, j+16) interleaved for
    the 4-byte d=2 gather granule); a plain [32, N] copy kept for the local
    self term.
  - Pooling: exact per-graph softmax. Nodes re-laid 4 graph-aligned quarters
    x 32 features across 128 partitions; gate/attn MLPs as block-diagonal
    128x128 fp16 matmuls; per-graph gate max via ap_gather into padded
    per-graph slots + 3D tensor_reduce; exp(gate - max) with the max
    broadcast back by a second tiny gather; denominator and weighted sums via
    per-quarter prefix scans probed at graph ends.
"""
import os
import sys

os.environ.setdefault("NEURON_RT_RESET_CORES", "1")
sys.path.insert(0, '/opt/trn_rl_repo')

import numpy as np


# -- NTFF profiling hook shim (optional; enables trace=True under axon) ----
def _install_ntff_shim():
    import types
    try:
        import antenv
        if 'antenv.axon_hooks' in sys.modules:
            return
        hooks = types.ModuleType('antenv.axon_hooks')
        _state = {'hook': None}
        hooks.set_axon_ntff_profile_hook = lambda h: _state.__setitem__('hook', h)
        hooks.get_axon_ntff_profile_hook = lambda: _state['hook']
        sys.modules['antenv.axon_hooks'] = hooks
        antenv.axon_hooks = hooks
        from trn_agent_boot.trn_boot import _ntff_profile_via_ctypes
        h = _ntff_profile_via_ctypes('/opt/axon/libaxon_pjrt.so')
        if h is not None:
            hooks.set_axon_ntff_profile_hook(h)
    except Exception:
        pass


_install_ntff_shim()

N_NODES = 262144
N_GRAPHS = 1024
C_IN = 16
H = 32
NC = 8
BLK = 32768
NCH = 1024                 # dst nodes per chunk
NCHUNK = 33
ECH = 2368                 # edge capacity per (block, chunk); max seen 2220
NMAX = NCH * NCHUNK        # 33792
QSTEP = NMAX // 4          # 8448: fixed quarter stride (same on all cores)
QW = 9728                  # 19*512: fixed quarter window width
QOFF = (0, QSTEP - 512, 2 * QSTEP - 512, 3 * QSTEP - 512)  # static offsets
H2PAD = QOFF[3] + QW - NMAX  # 768 zero-padded columns after h2
NSP = 24                   # graph slots per gmax gather pass
NS = 2 * NSP               # graph slots per quarter window (max seen 34)
SW = 320                   # slot width >= max graph size (max seen 317)
SENT = -1.0e5
TILE_N = 512
MAX_WAITS = 1

_cache = {}


def _split_multi_waits(nc, mybir, max_waits=MAX_WAITS):
    n_split = 0
    for fn in nc.m.functions:
        for bb in fn.blocks:
            out = []
            for ins in bb.instructions:
                si = ins.sync_info
                if si is not None and si.on_wait and len(si.on_wait) > max_waits:
                    waits = list(si.on_wait)
                    extra = waits[:-max_waits]
                    keep = waits[-max_waits:]
                    for i in range(0, len(extra), max_waits):
                        group = extra[i:i + max_waits]
                        nop = mybir.InstNoOp(
                            name=f"waitsplit_{nc.next_id()}",
                            sync_info=mybir.SyncInfo(on_wait=group, on_update=[]),
                            bass_nofuse=True,
                            engine=ins.engine,
                        )
                        out.append(nop)
                        n_split += 1
                    si.on_wait = keep
                out.append(ins)
            bb.instructions = out
    return n_split


def _wrap_idx(vals, group, arr, col0=0):
    """Wrapped ap_gather index layout: value i -> arr[16g + i%16, col0 + i//16]."""
    n = len(vals)
    assert n % 16 == 0
    v = np.asarray(vals, dtype=np.int16).reshape(n // 16, 16).T
    arr[16 * group:16 * group + 16, col0:col0 + n // 16] = v


def _register_cumsum():
    from concourse import dve_ops
    from concourse.dve_spec import Spec, Src0, C0, AluOp, lower
    import concourse.dve_spec as ds
    from concourse.dve_uop import DveOpSpec
    for op in dve_ops.OPS:
        if op.name == "CUMSUM_ANT":
            return op
    spec = Spec(
        body=ds.scan(AluOp.ADD, Src0, init=C0),
        reference=lambda in0, s0: np.cumsum(in0.astype(np.float32), axis=-1) + s0,
    )
    shas = {}
    for ver in ("v3", "v4"):
        uops = lower(spec, ver=ver)
        shas[ver] = DveOpSpec(name="CUMSUM_ANT", opcode=1, uops=uops,
                              rd1_en=False).sha(ver)
    op = dve_ops.DveOp("CUMSUM_ANT", spec, subdim=False, uops_sha=shas)
    dve_ops.OPS.append(op)
    dve_ops.CUSTOM_DVE_SPECS["CUMSUM_ANT"] = spec
    dve_ops._SUB_OPCODE_FOR_NAME["CUMSUM_ANT"] = \
        max(dve_ops._SUB_OPCODE_FOR_NAME.values()) + 1
    return op


# ================================================================ host prep
def _prep(edge_index, batch_vec):
    src = np.asarray(edge_index[0], dtype=np.int64)
    dst = np.asarray(edge_index[1], dtype=np.int64)
    bv = np.asarray(batch_vec, dtype=np.int64)

    gstart = np.searchsorted(bv, np.arange(N_GRAPHS))
    bounds = [0]
    for c in range(1, NC):
        target = c * (N_NODES // NC)
        gi = np.searchsorted(gstart, target)
        cand = []
        if gi < N_GRAPHS:
            cand.append(int(gstart[gi]))
        if gi > 0:
            cand.append(int(gstart[gi - 1]))
        bounds.append(min(cand, key=lambda v: abs(v - target)))
    bounds.append(N_NODES)
    n_lo = np.array(bounds[:-1])
    n_hi = np.array(bounds[1:])
    sizes = n_hi - n_lo
    assert sizes.max() <= NMAX, sizes
    g_lo = np.searchsorted(gstart, n_lo)
    g_hi = np.searchsorted(gstart, n_hi)

    owner = np.searchsorted(n_hi, dst, side='right')

    cores = []
    for c in range(NC):
        m = owner == c
        csrc = src[m]
        cdst_local = dst[m] - n_lo[c]
        size_c = int(sizes[c])

        ge = np.zeros((128, NCHUNK * ECH // 16), np.int16)
        gd = np.zeros((128, NCHUNK * NCH // 16), np.int16)

        blk_of = csrc >> 15
        src_local_all = (csrc & (BLK - 1))

        for k in range(NC):
            bm = blk_of == k
            bsrc = src_local_all[bm]
            bdst = cdst_local[bm]
            order = np.argsort(bdst, kind='stable')
            bsrc = bsrc[order].astype(np.int16)
            bdst = bdst[order]
            cnt = np.bincount(bdst, minlength=NMAX)
            cum = np.concatenate([[0], np.cumsum(cnt)])

            for ch in range(NCHUNK):
                a, b = ch * NCH, (ch + 1) * NCH
                e0, e1 = cum[a], cum[b]
                ne = int(e1 - e0)
                assert ne <= ECH, (c, k, ch, ne, ECH)
                ev = np.zeros(ECH, np.int16)
                ev[:ne] = bsrc[e0:e1]
                _wrap_idx(ev, k, ge, col0=ch * ECH // 16)
                ends = (cum[a + 1:b + 1] - e0).astype(np.int16)
                # transposed probe stream: slot s=b64*64+a64 -> node a64*16+b64
                # (decorrelates consecutive gather addresses)
                stream = ends.reshape(64, 16).T.flatten()
                _wrap_idx(stream, k, gd, col0=ch * NCH // 16)

        # ---- pooling quarter windows (fixed offsets, per-core content) ----
        glo, ghi = int(g_lo[c]), int(g_hi[c])
        ls = (gstart[glo:ghi] - n_lo[c]).astype(np.int64)       # graph starts
        le = np.concatenate([ls[1:], [size_c]]).astype(np.int64)  # graph ends
        q_of = ls // QSTEP  # quarter window owning each graph (by start node)

        gsa = np.zeros((128, NSP * SW // 16), np.int16)
        gsb = np.zeros((128, NSP * SW // 16), np.int16)
        gnode = np.zeros((128, QW // 16), np.int16)
        gend = np.zeros((128, NS // 16), np.int16)
        slot_map = np.full((4, NS), -1, np.int64)
        for q in range(4):
            off = QOFF[q]
            sel = np.where(q_of == q)[0]
            ngq = len(sel)
            assert ngq <= NS, (c, q, ngq)
            starts = ls[sel] - off
            ends = le[sel] - off
            lens = ends - starts
            if ngq:
                assert starts.min() >= 0 and ends.max() <= QW, (c, q)
                assert lens.max() <= SW, (c, q, lens.max())
            for s in range(ngq):
                slot_map[q, s] = int(sel[s])  # graph idx local to core

            # slot gather streams (two passes of NSP slots each); pads cycle
            # through the slot's own columns to avoid repeated-address stalls
            w_ar = np.arange(SW)
            for p, gs_arr in ((0, gsa), (1, gsb)):
                ev = np.empty(NSP * SW, np.int16)
                for si in range(NSP):
                    s = p * NSP + si
                    if s < ngq:
                        ev[si * SW:(si + 1) * SW] = \
                            (starts[s] + w_ar % lens[s]).astype(np.int16)
                    else:
                        ev[si * SW:(si + 1) * SW] = w_ar.astype(np.int16)
                _wrap_idx(ev, 2 * q, gs_arr)
                _wrap_idx(ev, 2 * q + 1, gs_arr)

            # node -> slot stream; foreign/pad columns -> slot NS (+1e5 max).
            # Stream order s -> node (s % 19) * 512 + s // 19 so consecutive
            # gather addresses land on different graphs.
            evn = np.full(QW, NS, np.int16)
            for s in range(ngq):
                evn[starts[s]:ends[s]] = s
            sidx = np.arange(QW)
            evn = evn[(sidx % (QW // TILE_N)) * TILE_N + sidx // (QW // TILE_N)]
            _wrap_idx(evn, 2 * q, gnode)
            _wrap_idx(evn, 2 * q + 1, gnode)

            # slot -> cumsum end-probe offset stream
            eve = np.zeros(NS, np.int16)
            prev = int(starts[0]) if ngq else 0
            for s in range(NS):
                if s < ngq:
                    prev = int(ends[s])
                eve[s] = prev
            _wrap_idx(eve, 2 * q, gend)
            _wrap_idx(eve, 2 * q + 1, gend)

        cores.append(dict(
            n_lo=int(n_lo[c]), size=size_c, g_lo=glo, g_hi=ghi,
            ge=ge, gd=gd, gsa=gsa, gsb=gsb, gnode=gnode, gend=gend,
            slot_map=slot_map,
        ))
    return cores, [int(b) for b in bounds]


# ================================================================ device
def _build_program(bounds):
    from concourse import bacc, tile
    from concourse.bass import mybir

    CUMSUM = _register_cumsum()

    f32 = mybir.dt.float32
    f16 = mybir.dt.float16
    i16 = mybir.dt.int16
    RELU = mybir.ActivationFunctionType.Relu
    EXP = mybir.ActivationFunctionType.Exp
    IDENT = mybir.ActivationFunctionType.Identity
    SUB = mybir.AluOpType.subtract
    MUL = mybir.AluOpType.mult

    nc = bacc.Bacc("TRN2", target_bir_lowering=False, debug=False, num_devices=NC)

    def din(name, shape, dt):
        return nc.dram_tensor(name, shape, dt, kind="ExternalInput")

    xt_in = din("xt", [128, BLK], f32)
    xo_in = din("xo", [16, NMAX], f16)
    ge_in = din("ge", [128, NCHUNK * ECH // 16], i16)
    gd_in = din("gd", [128, NCHUNK * NCH // 16], i16)
    gsa_in = din("gsa", [128, NSP * SW // 16], i16)
    gsb_in = din("gsb", [128, NSP * SW // 16], i16)
    gnode_in = din("gnode", [128, QW // 16], i16)
    gend_in = din("gend", [128, NS // 16], i16)
    wp16_in = din("wp16", [128, 1216], f16)
    wp32_in = din("wp32", [128, 16], f32)

    out_g = nc.dram_tensor("outg", [4, NS], f32, kind="ExternalOutput")

    h1i_own = nc.dram_tensor("h1i_own", [16, NMAX, 2], f16)
    h1i_all = nc.dram_tensor("h1i_all", [NC * 16, NMAX, 2], f16,
                             addr_space="Shared")
    h2_dram = nc.dram_tensor("h2d", [32, NMAX + H2PAD], f16)

    with tile.TileContext(nc) as tc:
        with (
            tc.tile_pool(name="cw", bufs=1) as cw,
        ):
            wp16 = cw.tile([128, 1216], f16, name="wp16")
            nc.sync.dma_start(wp16[:], wp16_in.ap()[:])
            wp32 = cw.tile([128, 16], f32, name="wp32")
            nc.sync.dma_start(wp32[:], wp32_in.ap()[:])

            zpad = cw.tile([32, H2PAD], f16, name="zpad")
            nc.vector.memset(zpad[:], 0.0)
            nc.sync.dma_start(h2_dram.ap()[:, NMAX:NMAX + H2PAD], zpad[:])

            w1blk_a = wp16[:, 0:16]
            w1blk_b = wp16[:, 16:32]
            w1s_a = wp16[0:16, 32:48]
            w1s_b = wp16[0:16, 48:64]
            w2blk0 = wp16[:, 64:96]
            w2blk1 = wp16[:, 96:128]
            w2s_a = wp16[0:16, 128:160]
            w2s_b = wp16[0:16, 160:192]
            BD = {}
            for i, nm in enumerate(("gw1", "gw2", "gw3r", "aw1", "aw2",
                                    "fw1", "fw2", "fw3r")):
                BD[nm] = wp16[:, 192 + 128 * i:192 + 128 * (i + 1)]
            b1a = wp32[0:16, 0:1]
            b1b = wp32[0:16, 1:2]
            b2f = wp32[0:32, 2:3]
            gb1t = wp32[:, 3:4]
            gb2t = wp32[:, 4:5]
            gb3t = wp32[:, 5:6]
            ab1t = wp32[:, 6:7]
            ab2t = wp32[:, 7:8]
            fb1t = wp32[:, 8:9]
            fb2t = wp32[:, 9:10]
            fb3t = wp32[:, 10:11]

            with (
                tc.tile_pool(name="tbl", bufs=1) as tblp,
            ):
                # ---------------- Layer 1 ----------------
                table1 = tblp.tile([128, BLK], f32, tag="table")
                nc.sync.dma_start(table1[:], xt_in.ap()[:])

                with nc.named_scope("L1"), (
                    tc.tile_pool(name="ip", bufs=4)) as ip, (
                    tc.tile_pool(name="xp", bufs=3)) as xp, (
                    tc.tile_pool(name="cp", bufs=2)) as cp, (
                    tc.tile_pool(name="gp", bufs=2)) as gp, (
                    tc.tile_pool(name="op", bufs=3)) as op, (
                    tc.tile_pool(name="pp", bufs=3, space="PSUM")) as pp:
                    for ch in range(NCHUNK):
                        gidx = ip.tile([128, ECH // 16], i16, tag="gidx")
                        nc.sync.dma_start(
                            gidx[:],
                            ge_in.ap()[:, ch * ECH // 16:(ch + 1) * ECH // 16])
                        didx = ip.tile([128, NCH // 16], i16, tag="didx")
                        nc.sync.dma_start(
                            didx[:],
                            gd_in.ap()[:, ch * NCH // 16:(ch + 1) * NCH // 16])
                        xoc = xp.tile([16, NCH], f16, tag="xoc")
                        nc.sync.dma_start(
                            xoc[:], xo_in.ap()[:, ch * NCH:(ch + 1) * NCH])

                        cs = cp.tile([128, 1 + ECH], f32, tag="cs")
                        nc.vector.memset(cs[:, 0:1], 0.0)
                        nc.gpsimd.ap_gather(
                            cs[:, 1:], table1[:], gidx[:],
                            channels=128, num_elems=BLK, d=1, num_idxs=ECH)
                        nc.vector._custom_dve(
                            CUMSUM, out=cs[:, 1:], in0=cs[:, 1:], s0=0.0)

                        # probes in transposed stream order: slot (b,a) holds
                        # cs[end of node a*16+b]
                        Gp = gp.tile([128, 16, 64], f32, tag="G")
                        nc.gpsimd.ap_gather(
                            Gp[:], cs[:], didx[:],
                            channels=128, num_elems=1 + ECH, d=1, num_idxs=NCH)
                        Pp = gp.tile([128, 16, 64], f16, tag="P")
                        nc.vector.tensor_tensor(Pp[:, 1:16, :], Gp[:, 1:16, :],
                                                Gp[:, 0:15, :], SUB)
                        nc.vector.tensor_tensor(Pp[:, 0, 1:64], Gp[:, 0, 1:64],
                                                Gp[:, 15, 0:63], SUB)
                        nc.vector.tensor_tensor(Pp[:, 0, 0:1], Gp[:, 0, 0:1],
                                                cs[:, 0:1], SUB)
                        PpT = Pp[:].transpose([0, 2, 1])  # [128, 64(a), 16(b)]

                        for t0 in range(0, NCH, TILE_N):
                            tn = TILE_N
                            a0 = t0 // 16
                            sl = slice(t0, t0 + tn)
                            rhsP = PpT[:, a0:a0 + 32, :]
                            pha = pp.tile([16, tn], f32, tag="psa")
                            nc.tensor.matmul(pha[:], w1blk_a, rhsP,
                                             start=True, stop=False)
                            nc.tensor.matmul(pha[:], w1s_a, xoc[:, sl],
                                             start=False, stop=True)
                            phb = pp.tile([16, tn], f32, tag="psb")
                            nc.tensor.matmul(phb[:], w1blk_b, rhsP,
                                             start=True, stop=False)
                            nc.tensor.matmul(phb[:], w1s_b, xoc[:, sl],
                                             start=False, stop=True)
                            he3 = op.tile([16, tn, 2], f16, tag="he3")
                            nc.scalar.activation(he3[:, :, 0], pha[:], RELU,
                                                 bias=b1a)
                            nc.scalar.activation(he3[:, :, 1], phb[:], RELU,
                                                 bias=b1b)
                            col = ch * NCH + t0
                            nc.sync.dma_start(
                                h1i_own.ap()[:, col:col + tn, :], he3[:])

                # ---------------- exchange ----------------
                with nc.named_scope("AG"):
                    nc.gpsimd.collective_compute(
                        "AllGather", mybir.AluOpType.bypass,
                        replica_groups=[list(range(NC))],
                        ins=[h1i_own.ap()[:]],
                        outs=[h1i_all.ap()[:]],
                    )

                # ---------------- table2 ----------------
                table2 = tblp.tile([128, BLK, 2], f16, tag="table")
                with nc.named_scope("T2"):
                    for k in range(NC):
                        lo, hi = k * BLK, (k + 1) * BLK
                        pos = lo
                        while pos < hi:
                            c2 = next(i for i in range(NC)
                                      if bounds[i] <= pos < bounds[i + 1])
                            seg_end = min(hi, bounds[c2 + 1])
                            ln = seg_end - pos
                            local = pos - bounds[c2]
                            nc.sync.dma_start(
                                table2[16 * k:16 * (k + 1),
                                       pos - lo:pos - lo + ln, :],
                                h1i_all.ap()[16 * c2:16 * (c2 + 1),
                                             local:local + ln, :])
                            pos = seg_end

                # ---------------- Layer 2 ----------------
                with nc.named_scope("L2"), (
                    tc.tile_pool(name="ip2", bufs=4)) as ip, (
                    tc.tile_pool(name="xp2", bufs=3)) as xp, (
                    tc.tile_pool(name="cp2", bufs=2)) as cp, (
                    tc.tile_pool(name="gp2", bufs=2)) as gp, (
                    tc.tile_pool(name="op2", bufs=3)) as op, (
                    tc.tile_pool(name="pp2", bufs=4, space="PSUM")) as pp:
                    for ch in range(NCHUNK):
                        gidx = ip.tile([128, ECH // 16], i16, tag="gidx")
                        nc.sync.dma_start(
                            gidx[:],
                            ge_in.ap()[:, ch * ECH // 16:(ch + 1) * ECH // 16])
                        didx = ip.tile([128, NCH // 16], i16, tag="didx")
                        nc.sync.dma_start(
                            didx[:],
                            gd_in.ap()[:, ch * NCH // 16:(ch + 1) * NCH // 16])
                        h1c = xp.tile([16, NCH, 2], f16, tag="h1c")
                        nc.sync.dma_start(
                            h1c[:], h1i_own.ap()[:, ch * NCH:(ch + 1) * NCH, :])

                        stage = cp.tile([128, ECH, 2], f16, tag="stage")
                        nc.gpsimd.ap_gather(
                            stage[:], table2[:], gidx[:],
                            channels=128, num_elems=BLK, d=2, num_idxs=ECH)
                        cs2 = cp.tile([128, 1 + ECH, 2], f32, tag="cs2", bufs=1)
                        nc.vector.memset(cs2[:, 0:1, :], 0.0)
                        nc.vector._custom_dve(
                            CUMSUM, out=cs2[:, 1:, 0], in0=stage[:, :, 0], s0=0.0)
                        nc.vector._custom_dve(
                            CUMSUM, out=cs2[:, 1:, 1], in0=stage[:, :, 1], s0=0.0)

                        G2 = gp.tile([128, 16, 64, 2], f32, tag="G2", bufs=1)
                        nc.gpsimd.ap_gather(
                            G2[:], cs2[:], didx[:],
                            channels=128, num_elems=1 + ECH, d=2, num_idxs=NCH)
                        P2 = gp.tile([128, 16, 64, 2], f16, tag="P2")
                        nc.vector.tensor_tensor(P2[:, 1:16, :, :],
                                                G2[:, 1:16, :, :],
                                                G2[:, 0:15, :, :], SUB)
                        nc.vector.tensor_tensor(P2[:, 0, 1:64, :],
                                                G2[:, 0, 1:64, :],
                                                G2[:, 15, 0:63, :], SUB)
                        nc.vector.tensor_tensor(P2[:, 0, 0:1, :],
                                                G2[:, 0, 0:1, :],
                                                cs2[:, 0:1, :], SUB)
                        P2T = P2[:].transpose([0, 2, 1, 3])  # [128,64,16,2]

                        for t0 in range(0, NCH, TILE_N):
                            tn = TILE_N
                            a0 = t0 // 16
                            sl = slice(t0, t0 + tn)
                            ph2 = pp.tile([H, tn], f32, tag="ps2")
                            nc.tensor.matmul(ph2[:], w2blk0,
                                             P2T[:, a0:a0 + 32, :, 0],
                                             start=True, stop=False)
                            nc.tensor.matmul(ph2[:], w2blk1,
                                             P2T[:, a0:a0 + 32, :, 1],
                                             start=False, stop=False)
                            nc.tensor.matmul(ph2[:], w2s_a, h1c[:, sl, 0],
                                             start=False, stop=False)
                            nc.tensor.matmul(ph2[:], w2s_b, h1c[:, sl, 1],
                                             start=False, stop=True)
                            h2t = op.tile([32, tn], f16, tag="h2t")
                            nc.scalar.activation(h2t[:], ph2[:], RELU, bias=b2f)
                            col = ch * NCH + t0
                            nc.sync.dma_start(
                                h2_dram.ap()[:, col:col + tn], h2t[:])

            # ---------------- pooling ----------------
            with (
                tc.tile_pool(name="p3", bufs=1) as p3,
                tc.tile_pool(name="tp", bufs=2) as tp,
                tc.tile_pool(name="pq", bufs=4, space="PSUM") as pq,
            ):
                gsa = p3.tile([128, NSP * SW // 16], i16)
                nc.sync.dma_start(gsa[:], gsa_in.ap()[:])
                gsb = p3.tile([128, NSP * SW // 16], i16)
                nc.sync.dma_start(gsb[:], gsb_in.ap()[:])
                gnode = p3.tile([128, QW // 16], i16)
                nc.sync.dma_start(gnode[:], gnode_in.ap()[:])
                gend = p3.tile([128, NS // 16], i16)
                nc.sync.dma_start(gend[:], gend_in.ap()[:])

                h2q = p3.tile([128, QW], f16)
                for q in range(4):
                    nc.sync.dma_start(
                        h2q[32 * q:32 * (q + 1), :],
                        h2_dram.ap()[:, QOFF[q]:QOFF[q] + QW])

                gate_q = p3.tile([128, QW + 1], f32)
                with nc.named_scope("PMLP"):
                    for t0 in range(0, QW, TILE_N):
                        sl = slice(t0, t0 + TILE_N)
                        pg1 = pq.tile([128, TILE_N], f32, tag="psq")
                        nc.tensor.matmul(pg1[:], BD["gw1"], h2q[:, sl],
                                         start=True, stop=True)
                        g1s = tp.tile([128, TILE_N], f16, tag="g1s")
                        nc.scalar.activation(g1s[:], pg1[:], RELU, bias=gb1t)
                        pg2 = pq.tile([128, TILE_N], f32, tag="psq")
                        nc.tensor.matmul(pg2[:], BD["gw2"], g1s[:],
                                         start=True, stop=True)
                        g2s = tp.tile([128, TILE_N], f16, tag="g2s")
                        nc.scalar.activation(g2s[:], pg2[:], RELU, bias=gb2t)
                        pg3 = pq.tile([128, TILE_N], f32, tag="psq")
                        nc.tensor.matmul(pg3[:], BD["gw3r"], g2s[:],
                                         start=True, stop=True)
                        nc.scalar.activation(gate_q[:, sl], pg3[:], IDENT,
                                             bias=gb3t)

                # sentinel column for empty-slot gathers
                nc.vector.memset(gate_q[:, QW:QW + 1], SENT)

                # per-graph max; extra slot NS = +1e5 kills foreign columns
                gmax = p3.tile([128, NS + 1], f32)
                nc.vector.memset(gmax[:, NS:NS + 1], 1.0e5)
                with nc.named_scope("GMAX"), \
                        tc.tile_pool(name="zp", bufs=1) as zp:
                    for p, gs_t in ((0, gsa), (1, gsb)):
                        Zs = zp.tile([128, NSP, SW], f32, tag="Zs")
                        nc.gpsimd.ap_gather(
                            Zs[:], gate_q[:], gs_t[:],
                            channels=128, num_elems=QW + 1, d=1,
                            num_idxs=NSP * SW)
                        nc.vector.tensor_reduce(
                            gmax[:, p * NSP:(p + 1) * NSP], Zs[:],
                            mybir.AxisListType.X, mybir.AluOpType.max)

                # second pass: attn mlp, exp, prefix sums
                csE = p3.tile([128, 1 + QW], f32)
                csW = p3.tile([128, 1 + QW], f32)
                nc.vector.memset(csE[:, 0:1], 0.0)
                nc.vector.memset(csW[:, 0:1], 0.0)
                NT = QW // TILE_N  # 19
                with nc.named_scope("PATT"), \
                        tc.tile_pool(name="mp", bufs=1) as mp:
                    # one decorrelated broadcast gather: col b*NT+a holds
                    # gmax[slot(node a*512+b)]
                    Mfull = mp.tile([128, TILE_N, NT], f32)
                    nc.gpsimd.ap_gather(
                        Mfull[:], gmax[:], gnode[:],
                        channels=128, num_elems=NS + 1, d=1, num_idxs=QW)
                    for t in range(NT):
                        t0 = t * TILE_N
                        sl = slice(t0, t0 + TILE_N)
                        pt1 = pq.tile([128, TILE_N], f32, tag="psq")
                        nc.tensor.matmul(pt1[:], BD["aw1"], h2q[:, sl],
                                         start=True, stop=True)
                        t1s = tp.tile([128, TILE_N], f16, tag="g1s")
                        nc.scalar.activation(t1s[:], pt1[:], RELU, bias=ab1t)
                        pt2 = pq.tile([128, TILE_N], f32, tag="psq")
                        nc.tensor.matmul(pt2[:], BD["aw2"], t1s[:],
                                         start=True, stop=True)
                        t2s = tp.tile([128, TILE_N], f32, tag="t2s")
                        nc.scalar.activation(t2s[:], pt2[:], RELU, bias=ab2t)

                        Ep = tp.tile([128, TILE_N], f32, tag="Ep")
                        nc.vector.tensor_tensor(Ep[:], gate_q[:, sl],
                                                Mfull[:, :, t], SUB)
                        Ee = tp.tile([128, TILE_N], f32, tag="Ee")
                        nc.scalar.activation(Ee[:], Ep[:], EXP)
                        wt = tp.tile([128, TILE_N], f32, tag="wt")
                        nc.vector.tensor_tensor(wt[:], Ee[:], t2s[:], MUL)
                        nc.vector._custom_dve(
                            CUMSUM, out=csE[:, 1 + t0:1 + t0 + TILE_N],
                            in0=Ee[:], s0=csE[:, t0:t0 + 1])
                        nc.vector._custom_dve(
                            CUMSUM, out=csW[:, 1 + t0:1 + t0 + TILE_N],
                            in0=wt[:], s0=csW[:, t0:t0 + 1])

                with nc.named_scope("PFIN"):
                    GdE = p3.tile([128, 1 + NS], f32)
                    GdW = p3.tile([128, 1 + NS], f32)
                    nc.vector.memset(GdE[:, 0:1], 0.0)
                    nc.vector.memset(GdW[:, 0:1], 0.0)
                    nc.gpsimd.ap_gather(
                        GdE[:, 1:], csE[:], gend[:],
                        channels=128, num_elems=1 + QW, d=1, num_idxs=NS)
                    nc.gpsimd.ap_gather(
                        GdW[:, 1:], csW[:], gend[:],
                        channels=128, num_elems=1 + QW, d=1, num_idxs=NS)
                    denom = p3.tile([128, NS], f32)
                    num = p3.tile([128, NS], f32)
                    nc.vector.tensor_tensor(denom[:], GdE[:, 1:], GdE[:, :-1],
                                            SUB)
                    nc.vector.tensor_tensor(num[:], GdW[:, 1:], GdW[:, :-1], SUB)
                    nc.vector.tensor_scalar_max(denom[:], denom[:], 1e-16)
                    rec = p3.tile([128, NS], f32)
                    nc.vector.reciprocal(rec[:], denom[:])
                    pooled = p3.tile([128, NS], f16)
                    nc.vector.tensor_tensor(pooled[:], num[:], rec[:], MUL)

                    pc1 = pq.tile([128, NS], f32, tag="psc", bufs=2)
                    nc.tensor.matmul(pc1[:], BD["fw1"], pooled[:],
                                     start=True, stop=True)
                    c1s = p3.tile([128, NS], f16)
                    nc.scalar.activation(c1s[:], pc1[:], RELU, bias=fb1t)
                    pc2 = pq.tile([128, NS], f32, tag="psc", bufs=2)
                    nc.tensor.matmul(pc2[:], BD["fw2"], c1s[:],
                                     start=True, stop=True)
                    c2s = p3.tile([128, NS], f16)
                    nc.scalar.activation(c2s[:], pc2[:], RELU, bias=fb2t)
                    pc3 = pq.tile([128, NS], f32, tag="psc", bufs=2)
                    nc.tensor.matmul(pc3[:], BD["fw3r"], c2s[:],
                                     start=True, stop=True)
                    o3 = p3.tile([128, NS], f32)
                    nc.scalar.activation(o3[:], pc3[:], IDENT, bias=fb3t)
                    for q in range(4):
                        nc.sync.dma_start(out_g.ap()[q:q + 1, :],
                                          o3[32 * q:32 * q + 1, :])

    nc.compile()
    _split_multi_waits(nc, mybir)
    return nc


# ================================================================ entry
def kernel(x, w1, b1, w2, b2, gw1, gb1, gw2, gb2, gw3, gb3,
           aw1, ab1, aw2, ab2, fw1, fb1, fw2, fb2, fw3, fb3,
           edge_index, batch_vec, num_graphs):
    from concourse.bass_utils import run_bass_kernel_spmd

    x = np.asarray(x, np.float32)
    cores, bounds = _prep(edge_index, batch_vec)

    f32a = lambda a: np.asarray(a, np.float32)
    f16a = lambda a: np.ascontiguousarray(np.asarray(a, np.float32)
                                          .astype(np.float16))

    w1n, w2n = f32a(w1), f32a(w2)

    xt = np.zeros((128, BLK), np.float32)
    for k in range(NC):
        xt[16 * k:16 * (k + 1), :] = x[BLK * k:BLK * (k + 1), :].T

    def bd4(w):
        out = np.zeros((128, 128), np.float32)
        for q in range(4):
            out[32 * q:32 * (q + 1), 32 * q:32 * (q + 1)] = w
        return out

    gw3r = np.tile(f32a(gw3).reshape(H, 1), (1, H))
    fw3r = np.tile(f32a(fw3).reshape(H, 1), (1, H))

    wp16 = np.zeros((128, 1216), np.float32)
    wp16[:, 0:16] = np.tile(w1n[:, 0:16], (8, 1))
    wp16[:, 16:32] = np.tile(w1n[:, 16:32], (8, 1))
    wp16[0:16, 32:48] = w1n[:, 0:16]
    wp16[0:16, 48:64] = w1n[:, 16:32]
    wp16[:, 64:96] = np.tile(w2n[0:16, :], (8, 1))
    wp16[:, 96:128] = np.tile(w2n[16:32, :], (8, 1))
    wp16[0:16, 128:160] = w2n[0:16, :]
    wp16[0:16, 160:192] = w2n[16:32, :]
    for i, w in enumerate((gw1, gw2, gw3r, aw1, aw2, fw1, fw2, fw3r)):
        wp16[:, 192 + 128 * i:192 + 128 * (i + 1)] = bd4(f32a(w))
    wp16 = wp16.astype(np.float16)

    def t4(b):
        return np.tile(f32a(b).reshape(H), 4)

    wp32 = np.zeros((128, 16), np.float32)
    wp32[0:16, 0] = f32a(b1).reshape(-1)[0:16]
    wp32[0:16, 1] = f32a(b1).reshape(-1)[16:32]
    wp32[0:32, 2] = f32a(b2).reshape(-1)
    wp32[:, 3] = t4(gb1)
    wp32[:, 4] = t4(gb2)
    wp32[:, 5] = float(np.asarray(gb3).reshape(-1)[0])
    wp32[:, 6] = t4(ab1)
    wp32[:, 7] = t4(ab2)
    wp32[:, 8] = t4(fb1)
    wp32[:, 9] = t4(fb2)
    wp32[:, 10] = float(np.asarray(fb3).reshape(-1)[0])

    common = dict(xt=xt, wp16=wp16, wp32=wp32)

    in_maps = []
    for c, info in enumerate(cores):
        xo = np.zeros((16, NMAX), np.float16)
        xo[:, :info['size']] = \
            x[info['n_lo']:info['n_lo'] + info['size'], :].T.astype(np.float16)
        m = dict(common)
        m.update(xo=xo, ge=info['ge'], gd=info['gd'], gsa=info['gsa'],
                 gsb=info['gsb'], gnode=info['gnode'], gend=info['gend'])
        in_maps.append(m)

    key = tuple(bounds)
    if _cache.get('key') != key:
        _cache['nc'] = _build_program(bounds)
        _cache['key'] = key
    ncp = _cache['nc']

    res = run_bass_kernel_spmd(ncp, in_maps, core_ids=list(range(NC)),
                               trace=bool(os.environ.get("KERNEL_TRACE")))
    _cache['last_results'] = res

    out = np.zeros((N_GRAPHS, 1), np.float32)
    for c, info in enumerate(cores):
        vals = np.asarray(res.results[c]["outg"])  # [4, NS]
        for q in range(4):
            for s in range(NS):
                g = info['slot_map'][q, s]
                if g >= 0:
                    out[info['g_lo'] + g, 0] = vals[q, s]
    return out


# revision 37
# speedup vs baseline: 1.2581x; 1.0599x over previous
"""2-layer GIN + attentional pooling on 8 Trainium2 NeuronCores (Bass/Tile).

v2 rewrite of the gather/cumsum baseline:
  - Nodes split into 8 graph-aligned ownership ranges (one per core); each
    core processes edges whose dst it owns, bucketed by 32768-node src block
    (one per GPSIMD core-group) and dst-ordered within 1024-node chunks.
  - Segment sums over dst: DVE prefix-scan over the dst-sorted edge stream,
    ap_gather of the cumsum at host-known segment ends, shifted subtraction.
  - GIN linear fused into the block fold: (x + A.x) @ w == w_blk-fold(P) + w.x,
    so each 512-node tile is 2 (L1) / 3 (L2) fp16 matmuls, single PE pass.
  - h1 exchanged with AllGather (f16, feature pairs (j, j+16) interleaved for
    the 4-byte d=2 gather granule); a plain [32, N] copy kept for the local
    self term.
  - Pooling: exact per-graph softmax. Nodes re-laid 4 graph-aligned quarters
    x 32 features across 128 partitions; gate/attn MLPs as block-diagonal
    128x128 fp16 matmuls; per-graph gate max via ap_gather into padded
    per-graph slots + 3D tensor_reduce; exp(gate - max) with the max
    broadcast back by a second tiny gather; denominator and weighted sums via
    per-quarter prefix scans probed at graph ends.
"""
import os
import sys

os.environ.setdefault("NEURON_RT_RESET_CORES", "1")
sys.path.insert(0, '/opt/trn_rl_repo')

import numpy as np


# -- NTFF profiling hook shim (optional; enables trace=True under axon) ----
def _install_ntff_shim():
    import types
    try:
        import antenv
        if 'antenv.axon_hooks' in sys.modules:
            return
        hooks = types.ModuleType('antenv.axon_hooks')
        _state = {'hook': None}
        hooks.set_axon_ntff_profile_hook = lambda h: _state.__setitem__('hook', h)
        hooks.get_axon_ntff_profile_hook = lambda: _state['hook']
        sys.modules['antenv.axon_hooks'] = hooks
        antenv.axon_hooks = hooks
        from trn_agent_boot.trn_boot import _ntff_profile_via_ctypes
        h = _ntff_profile_via_ctypes('/opt/axon/libaxon_pjrt.so')
        if h is not None:
            hooks.set_axon_ntff_profile_hook(h)
    except Exception:
        pass


_install_ntff_shim()

N_NODES = 262144
N_GRAPHS = 1024
C_IN = 16
H = 32
NC = 8
BLK = 32768
NCH = 1024                 # dst nodes per chunk
NCHUNK = 33
ECH = 2368                 # edge capacity per (block, chunk); max seen 2220
NMAX = NCH * NCHUNK        # 33792
QSTEP = NMAX // 4          # 8448: fixed quarter stride (same on all cores)
QW = 9728                  # 19*512: fixed quarter window width
QOFF = (0, QSTEP - 512, 2 * QSTEP - 512, 3 * QSTEP - 512)  # static offsets
H2PAD = QOFF[3] + QW - NMAX  # 768 zero-padded columns after h2
NSP = 24                   # graph slots per gmax gather pass
NS = 2 * NSP               # graph slots per quarter window (max seen 34)
SW = 320                   # slot width >= max graph size (max seen 317)
SENT = -1.0e5
TILE_N = 512
MAX_WAITS = 1

_cache = {}


def _split_multi_waits(nc, mybir, max_waits=MAX_WAITS):
    n_split = 0
    for fn in nc.m.functions:
        for bb in fn.blocks:
            out = []
            for ins in bb.instructions:
                si = ins.sync_info
                if si is not None and si.on_wait and len(si.on_wait) > max_waits:
                    waits = list(si.on_wait)
                    extra = waits[:-max_waits]
                    keep = waits[-max_waits:]
                    for i in range(0, len(extra), max_waits):
                        group = extra[i:i + max_waits]
                        nop = mybir.InstNoOp(
                            name=f"waitsplit_{nc.next_id()}",
                            sync_info=mybir.SyncInfo(on_wait=group, on_update=[]),
                            bass_nofuse=True,
                            engine=ins.engine,
                        )
                        out.append(nop)
                        n_split += 1
                    si.on_wait = keep
                out.append(ins)
            bb.instructions = out
    return n_split


def _wrap_idx(vals, group, arr, col0=0):
    """Wrapped ap_gather index layout: value i -> arr[16g + i%16, col0 + i//16]."""
    n = len(vals)
    assert n % 16 == 0
    v = np.asarray(vals, dtype=np.int16).reshape(n // 16, 16).T
    arr[16 * group:16 * group + 16, col0:col0 + n // 16] = v


def _register_cumsum():
    from concourse import dve_ops
    from concourse.dve_spec import Spec, Src0, C0, AluOp, lower
    import concourse.dve_spec as ds
    from concourse.dve_uop import DveOpSpec
    for op in dve_ops.OPS:
        if op.name == "CUMSUM_ANT":
            return op
    spec = Spec(
        body=ds.scan(AluOp.ADD, Src0, init=C0),
        reference=lambda in0, s0: np.cumsum(in0.astype(np.float32), axis=-1) + s0,
    )
    shas = {}
    for ver in ("v3", "v4"):
        uops = lower(spec, ver=ver)
        shas[ver] = DveOpSpec(name="CUMSUM_ANT", opcode=1, uops=uops,
                              rd1_en=False).sha(ver)
    op = dve_ops.DveOp("CUMSUM_ANT", spec, subdim=False, uops_sha=shas)
    dve_ops.OPS.append(op)
    dve_ops.CUSTOM_DVE_SPECS["CUMSUM_ANT"] = spec
    dve_ops._SUB_OPCODE_FOR_NAME["CUMSUM_ANT"] = \
        max(dve_ops._SUB_OPCODE_FOR_NAME.values()) + 1
    return op


# ================================================================ host prep
def _prep(edge_index, batch_vec):
    src = np.asarray(edge_index[0], dtype=np.int64)
    dst = np.asarray(edge_index[1], dtype=np.int64)
    bv = np.asarray(batch_vec, dtype=np.int64)

    gstart = np.searchsorted(bv, np.arange(N_GRAPHS))
    bounds = [0]
    for c in range(1, NC):
        target = c * (N_NODES // NC)
        gi = np.searchsorted(gstart, target)
        cand = []
        if gi < N_GRAPHS:
            cand.append(int(gstart[gi]))
        if gi > 0:
            cand.append(int(gstart[gi - 1]))
        bounds.append(min(cand, key=lambda v: abs(v - target)))
    bounds.append(N_NODES)
    n_lo = np.array(bounds[:-1])
    n_hi = np.array(bounds[1:])
    sizes = n_hi - n_lo
    assert sizes.max() <= NMAX, sizes
    g_lo = np.searchsorted(gstart, n_lo)
    g_hi = np.searchsorted(gstart, n_hi)

    owner = np.searchsorted(n_hi, dst, side='right')

    nvi = np.zeros(NCHUNK, np.int64)  # per-chunk edge gather length (all cores)
    cores = []
    for c in range(NC):
        m = owner == c
        csrc = src[m]
        cdst_local = dst[m] - n_lo[c]
        size_c = int(sizes[c])

        ge = np.zeros((128, NCHUNK * ECH // 16), np.int16)
        gd = np.zeros((128, NCHUNK * NCH // 16), np.int16)

        blk_of = csrc >> 15
        src_local_all = (csrc & (BLK - 1))

        for k in range(NC):
            bm = blk_of == k
            bsrc = src_local_all[bm]
            bdst = cdst_local[bm]
            order = np.argsort(bdst, kind='stable')
            bsrc = bsrc[order].astype(np.int16)
            bdst = bdst[order]
            cnt = np.bincount(bdst, minlength=NMAX)
            cum = np.concatenate([[0], np.cumsum(cnt)])

            for ch in range(NCHUNK):
                a, b = ch * NCH, (ch + 1) * NCH
                e0, e1 = cum[a], cum[b]
                ne = int(e1 - e0)
                assert ne <= ECH, (c, k, ch, ne, ECH)
                nvi[ch] = max(nvi[ch], -(-ne // 16) * 16)
                ev = np.zeros(ECH, np.int16)
                ev[:ne] = bsrc[e0:e1]
                _wrap_idx(ev, k, ge, col0=ch * ECH // 16)
                ends = (cum[a + 1:b + 1] - e0).astype(np.int16)
                # transposed probe stream: slot s=b64*64+a64 -> node a64*16+b64
                # (decorrelates consecutive gather addresses)
                stream = ends.reshape(64, 16).T.flatten()
                _wrap_idx(stream, k, gd, col0=ch * NCH // 16)

        # ---- pooling quarter windows (fixed offsets, per-core content) ----
        glo, ghi = int(g_lo[c]), int(g_hi[c])
        ls = (gstart[glo:ghi] - n_lo[c]).astype(np.int64)       # graph starts
        le = np.concatenate([ls[1:], [size_c]]).astype(np.int64)  # graph ends
        q_of = ls // QSTEP  # quarter window owning each graph (by start node)

        gsa = np.zeros((128, NSP * SW // 16), np.int16)
        gsb = np.zeros((128, NSP * SW // 16), np.int16)
        gnode = np.zeros((128, QW // 16), np.int16)
        gend = np.zeros((128, NS // 16), np.int16)
        slot_map = np.full((4, NS), -1, np.int64)
        for q in range(4):
            off = QOFF[q]
            sel = np.where(q_of == q)[0]
            ngq = len(sel)
            assert ngq <= NS, (c, q, ngq)
            starts = ls[sel] - off
            ends = le[sel] - off
            lens = ends - starts
            if ngq:
                assert starts.min() >= 0 and ends.max() <= QW, (c, q)
                assert lens.max() <= SW, (c, q, lens.max())
            for s in range(ngq):
                slot_map[q, s] = int(sel[s])  # graph idx local to core

            # slot gather streams (two passes of NSP slots each); pads cycle
            # through the slot's own columns to avoid repeated-address stalls
            w_ar = np.arange(SW)
            for p, gs_arr in ((0, gsa), (1, gsb)):
                ev = np.empty(NSP * SW, np.int16)
                for si in range(NSP):
                    s = p * NSP + si
                    if s < ngq:
                        ev[si * SW:(si + 1) * SW] = \
                            (starts[s] + w_ar % lens[s]).astype(np.int16)
                    else:
                        ev[si * SW:(si + 1) * SW] = w_ar.astype(np.int16)
                _wrap_idx(ev, 2 * q, gs_arr)
                _wrap_idx(ev, 2 * q + 1, gs_arr)

            # node -> slot stream; foreign/pad columns -> slot NS (+1e5 max).
            # Stream order s -> node (s % 19) * 512 + s // 19 so consecutive
            # gather addresses land on different graphs.
            evn = np.full(QW, NS, np.int16)
            for s in range(ngq):
                evn[starts[s]:ends[s]] = s
            sidx = np.arange(QW)
            evn = evn[(sidx % (QW // TILE_N)) * TILE_N + sidx // (QW // TILE_N)]
            _wrap_idx(evn, 2 * q, gnode)
            _wrap_idx(evn, 2 * q + 1, gnode)

            # slot -> cumsum end-probe offset stream
            eve = np.zeros(NS, np.int16)
            prev = int(starts[0]) if ngq else 0
            for s in range(NS):
                if s < ngq:
                    prev = int(ends[s])
                eve[s] = prev
            _wrap_idx(eve, 2 * q, gend)
            _wrap_idx(eve, 2 * q + 1, gend)

        cores.append(dict(
            n_lo=int(n_lo[c]), size=size_c, g_lo=glo, g_hi=ghi,
            ge=ge, gd=gd, gsa=gsa, gsb=gsb, gnode=gnode, gend=gend,
            slot_map=slot_map,
        ))
    return cores, [int(b) for b in bounds], [int(v) for v in nvi]


# ================================================================ device
def _build_program(bounds, nvi):
    from concourse import bacc, tile
    from concourse.bass import mybir

    CUMSUM = _register_cumsum()

    f32 = mybir.dt.float32
    f16 = mybir.dt.float16
    i16 = mybir.dt.int16
    RELU = mybir.ActivationFunctionType.Relu
    EXP = mybir.ActivationFunctionType.Exp
    IDENT = mybir.ActivationFunctionType.Identity
    SUB = mybir.AluOpType.subtract
    MUL = mybir.AluOpType.mult

    nc = bacc.Bacc("TRN2", target_bir_lowering=False, debug=False, num_devices=NC)

    def din(name, shape, dt):
        return nc.dram_tensor(name, shape, dt, kind="ExternalInput")

    xt_in = din("xt", [128, BLK], f32)
    xo_in = din("xo", [16, NMAX], f16)
    ge_in = din("ge", [128, NCHUNK * ECH // 16], i16)
    gd_in = din("gd", [128, NCHUNK * NCH // 16], i16)
    gsa_in = din("gsa", [128, NSP * SW // 16], i16)
    gsb_in = din("gsb", [128, NSP * SW // 16], i16)
    gnode_in = din("gnode", [128, QW // 16], i16)
    gend_in = din("gend", [128, NS // 16], i16)
    wp16_in = din("wp16", [128, 1216], f16)
    wp32_in = din("wp32", [128, 16], f32)

    out_g = nc.dram_tensor("outg", [4, NS], f32, kind="ExternalOutput")

    h1i_own = nc.dram_tensor("h1i_own", [16, NMAX, 2], f16)
    h1i_all = nc.dram_tensor("h1i_all", [NC * 16, NMAX, 2], f16,
                             addr_space="Shared")
    h2_dram = nc.dram_tensor("h2d", [32, NMAX + H2PAD], f16)

    with tile.TileContext(nc) as tc:
        with (
            tc.tile_pool(name="cw", bufs=1) as cw,
        ):
            wp16 = cw.tile([128, 1216], f16, name="wp16")
            nc.sync.dma_start(wp16[:], wp16_in.ap()[:])
            wp32 = cw.tile([128, 16], f32, name="wp32")
            nc.sync.dma_start(wp32[:], wp32_in.ap()[:])

            zpad = cw.tile([32, H2PAD], f16, name="zpad")
            nc.vector.memset(zpad[:], 0.0)
            nc.sync.dma_start(h2_dram.ap()[:, NMAX:NMAX + H2PAD], zpad[:])

            w1blk_a = wp16[:, 0:16]
            w1blk_b = wp16[:, 16:32]
            w1s_a = wp16[0:16, 32:48]
            w1s_b = wp16[0:16, 48:64]
            w2blk0 = wp16[:, 64:96]
            w2blk1 = wp16[:, 96:128]
            w2s_a = wp16[0:16, 128:160]
            w2s_b = wp16[0:16, 160:192]
            BD = {}
            for i, nm in enumerate(("gw1", "gw2", "gw3r", "aw1", "aw2",
                                    "fw1", "fw2", "fw3r")):
                BD[nm] = wp16[:, 192 + 128 * i:192 + 128 * (i + 1)]
            b1a = wp32[0:16, 0:1]
            b1b = wp32[0:16, 1:2]
            b2f = wp32[0:32, 2:3]
            gb1t = wp32[:, 3:4]
            gb2t = wp32[:, 4:5]
            gb3t = wp32[:, 5:6]
            ab1t = wp32[:, 6:7]
            ab2t = wp32[:, 7:8]
            fb1t = wp32[:, 8:9]
            fb2t = wp32[:, 9:10]
            fb3t = wp32[:, 10:11]

            with (
                tc.tile_pool(name="tbl", bufs=1) as tblp,
            ):
                # ---------------- Layer 1 ----------------
                table1 = tblp.tile([128, BLK], f32, tag="table")
                nc.sync.dma_start(table1[:], xt_in.ap()[:])

                with nc.named_scope("L1"), (
                    tc.tile_pool(name="ip", bufs=4)) as ip, (
                    tc.tile_pool(name="xp", bufs=3)) as xp, (
                    tc.tile_pool(name="cp", bufs=2)) as cp, (
                    tc.tile_pool(name="gp", bufs=2)) as gp, (
                    tc.tile_pool(name="op", bufs=3)) as op, (
                    tc.tile_pool(name="pp", bufs=3, space="PSUM")) as pp:
                    for ch in range(NCHUNK):
                        n = nvi[ch]
                        gidx = ip.tile([128, ECH // 16], i16, tag="gidx")
                        nc.sync.dma_start(
                            gidx[:, :n // 16],
                            ge_in.ap()[:, ch * ECH // 16:
                                       ch * ECH // 16 + n // 16])
                        didx = ip.tile([128, NCH // 16], i16, tag="didx")
                        nc.sync.dma_start(
                            didx[:],
                            gd_in.ap()[:, ch * NCH // 16:(ch + 1) * NCH // 16])
                        xoc = xp.tile([16, NCH], f16, tag="xoc")
                        nc.sync.dma_start(
                            xoc[:], xo_in.ap()[:, ch * NCH:(ch + 1) * NCH])

                        cs = cp.tile([128, 1 + ECH], f32, tag="cs")
                        nc.vector.memset(cs[:, 0:1], 0.0)
                        nc.gpsimd.ap_gather(
                            cs[:, 1:1 + n], table1[:], gidx[:, :n // 16],
                            channels=128, num_elems=BLK, d=1, num_idxs=n)
                        nc.vector._custom_dve(
                            CUMSUM, out=cs[:, 1:1 + n], in0=cs[:, 1:1 + n],
                            s0=0.0)

                        # probes in transposed stream order: slot (b,a) holds
                        # cs[end of node a*16+b]
                        Gp = gp.tile([128, 16, 64], f32, tag="G")
                        nc.gpsimd.ap_gather(
                            Gp[:], cs[:], didx[:],
                            channels=128, num_elems=1 + ECH, d=1, num_idxs=NCH)
                        Pp = gp.tile([128, 16, 64], f16, tag="P")
                        nc.vector.tensor_tensor(Pp[:, 1:16, :], Gp[:, 1:16, :],
                                                Gp[:, 0:15, :], SUB)
                        nc.vector.tensor_tensor(Pp[:, 0, 1:64], Gp[:, 0, 1:64],
                                                Gp[:, 15, 0:63], SUB)
                        nc.vector.tensor_tensor(Pp[:, 0, 0:1], Gp[:, 0, 0:1],
                                                cs[:, 0:1], SUB)
                        PpT = Pp[:].transpose([0, 2, 1])  # [128, 64(a), 16(b)]

                        for t0 in range(0, NCH, TILE_N):
                            tn = TILE_N
                            a0 = t0 // 16
                            sl = slice(t0, t0 + tn)
                            rhsP = PpT[:, a0:a0 + 32, :]
                            pha = pp.tile([16, tn], f32, tag="psa")
                            nc.tensor.matmul(pha[:], w1blk_a, rhsP,
                                             start=True, stop=False)
                            nc.tensor.matmul(pha[:], w1s_a, xoc[:, sl],
                                             start=False, stop=True)
                            phb = pp.tile([16, tn], f32, tag="psb")
                            nc.tensor.matmul(phb[:], w1blk_b, rhsP,
                                             start=True, stop=False)
                            nc.tensor.matmul(phb[:], w1s_b, xoc[:, sl],
                                             start=False, stop=True)
                            he3 = op.tile([16, tn, 2], f16, tag="he3")
                            nc.scalar.activation(he3[:, :, 0], pha[:], RELU,
                                                 bias=b1a)
                            nc.scalar.activation(he3[:, :, 1], phb[:], RELU,
                                                 bias=b1b)
                            col = ch * NCH + t0
                            nc.sync.dma_start(
                                h1i_own.ap()[:, col:col + tn, :], he3[:])

                # ---------------- exchange ----------------
                with nc.named_scope("AG"):
                    nc.gpsimd.collective_compute(
                        "AllGather", mybir.AluOpType.bypass,
                        replica_groups=[list(range(NC))],
                        ins=[h1i_own.ap()[:]],
                        outs=[h1i_all.ap()[:]],
                    )

                # ---------------- table2 ----------------
                table2 = tblp.tile([128, BLK, 2], f16, tag="table")
                with nc.named_scope("T2"):
                    for k in range(NC):
                        lo, hi = k * BLK, (k + 1) * BLK
                        pos = lo
                        while pos < hi:
                            c2 = next(i for i in range(NC)
                                      if bounds[i] <= pos < bounds[i + 1])
                            seg_end = min(hi, bounds[c2 + 1])
                            ln = seg_end - pos
                            local = pos - bounds[c2]
                            nc.sync.dma_start(
                                table2[16 * k:16 * (k + 1),
                                       pos - lo:pos - lo + ln, :],
                                h1i_all.ap()[16 * c2:16 * (c2 + 1),
                                             local:local + ln, :])
                            pos = seg_end

                # ---------------- Layer 2 ----------------
                with nc.named_scope("L2"), (
                    tc.tile_pool(name="ip2", bufs=4)) as ip, (
                    tc.tile_pool(name="xp2", bufs=3)) as xp, (
                    tc.tile_pool(name="cp2", bufs=2)) as cp, (
                    tc.tile_pool(name="gp2", bufs=2)) as gp, (
                    tc.tile_pool(name="op2", bufs=3)) as op, (
                    tc.tile_pool(name="pp2", bufs=4, space="PSUM")) as pp:
                    for ch in range(NCHUNK):
                        n = nvi[ch]
                        gidx = ip.tile([128, ECH // 16], i16, tag="gidx")
                        nc.sync.dma_start(
                            gidx[:, :n // 16],
                            ge_in.ap()[:, ch * ECH // 16:
                                       ch * ECH // 16 + n // 16])
                        didx = ip.tile([128, NCH // 16], i16, tag="didx")
                        nc.sync.dma_start(
                            didx[:],
                            gd_in.ap()[:, ch * NCH // 16:(ch + 1) * NCH // 16])
                        h1c = xp.tile([16, NCH, 2], f16, tag="h1c")
                        nc.sync.dma_start(
                            h1c[:], h1i_own.ap()[:, ch * NCH:(ch + 1) * NCH, :])

                        stage = cp.tile([128, ECH, 2], f16, tag="stage")
                        nc.gpsimd.ap_gather(
                            stage[:, :n, :], table2[:], gidx[:, :n // 16],
                            channels=128, num_elems=BLK, d=2, num_idxs=n)
                        cs2 = cp.tile([128, 1 + ECH, 2], f32, tag="cs2", bufs=1)
                        nc.vector.memset(cs2[:, 0:1, :], 0.0)
                        nc.vector._custom_dve(
                            CUMSUM, out=cs2[:, 1:1 + n, 0], in0=stage[:, :n, 0],
                            s0=0.0)
                        nc.vector._custom_dve(
                            CUMSUM, out=cs2[:, 1:1 + n, 1], in0=stage[:, :n, 1],
                            s0=0.0)

                        G2 = gp.tile([128, 16, 64, 2], f32, tag="G2", bufs=1)
                        nc.gpsimd.ap_gather(
                            G2[:], cs2[:], didx[:],
                            channels=128, num_elems=1 + ECH, d=2, num_idxs=NCH)
                        P2 = gp.tile([128, 16, 64, 2], f16, tag="P2")
                        nc.vector.tensor_tensor(P2[:, 1:16, :, :],
                                                G2[:, 1:16, :, :],
                                                G2[:, 0:15, :, :], SUB)
                        nc.vector.tensor_tensor(P2[:, 0, 1:64, :],
                                                G2[:, 0, 1:64, :],
                                                G2[:, 15, 0:63, :], SUB)
                        nc.vector.tensor_tensor(P2[:, 0, 0:1, :],
                                                G2[:, 0, 0:1, :],
                                                cs2[:, 0:1, :], SUB)
                        P2T = P2[:].transpose([0, 2, 1, 3])  # [128,64,16,2]

                        for t0 in range(0, NCH, TILE_N):
                            tn = TILE_N
                            a0 = t0 // 16
                            sl = slice(t0, t0 + tn)
                            ph2 = pp.tile([H, tn], f32, tag="ps2")
                            nc.tensor.matmul(ph2[:], w2blk0,
                                             P2T[:, a0:a0 + 32, :, 0],
                                             start=True, stop=False)
                            nc.tensor.matmul(ph2[:], w2blk1,
                                             P2T[:, a0:a0 + 32, :, 1],
                                             start=False, stop=False)
                            nc.tensor.matmul(ph2[:], w2s_a, h1c[:, sl, 0],
                                             start=False, stop=False)
                            nc.tensor.matmul(ph2[:], w2s_b, h1c[:, sl, 1],
                                             start=False, stop=True)
                            h2t = op.tile([32, tn], f16, tag="h2t")
                            nc.scalar.activation(h2t[:], ph2[:], RELU, bias=b2f)
                            col = ch * NCH + t0
                            nc.sync.dma_start(
                                h2_dram.ap()[:, col:col + tn], h2t[:])

            # ---------------- pooling ----------------
            with (
                tc.tile_pool(name="p3", bufs=1) as p3,
                tc.tile_pool(name="tp", bufs=2) as tp,
                tc.tile_pool(name="pq", bufs=4, space="PSUM") as pq,
            ):
                gsa = p3.tile([128, NSP * SW // 16], i16)
                nc.sync.dma_start(gsa[:], gsa_in.ap()[:])
                gsb = p3.tile([128, NSP * SW // 16], i16)
                nc.sync.dma_start(gsb[:], gsb_in.ap()[:])
                gnode = p3.tile([128, QW // 16], i16)
                nc.sync.dma_start(gnode[:], gnode_in.ap()[:])
                gend = p3.tile([128, NS // 16], i16)
                nc.sync.dma_start(gend[:], gend_in.ap()[:])

                h2q = p3.tile([128, QW], f16)
                for q in range(4):
                    nc.sync.dma_start(
                        h2q[32 * q:32 * (q + 1), :],
                        h2_dram.ap()[:, QOFF[q]:QOFF[q] + QW])

                gate_q = p3.tile([128, QW + 1], f32)
                with nc.named_scope("PMLP"):
                    for t0 in range(0, QW, TILE_N):
                        sl = slice(t0, t0 + TILE_N)
                        pg1 = pq.tile([128, TILE_N], f32, tag="psq")
                        nc.tensor.matmul(pg1[:], BD["gw1"], h2q[:, sl],
                                         start=True, stop=True)
                        g1s = tp.tile([128, TILE_N], f16, tag="g1s")
                        nc.scalar.activation(g1s[:], pg1[:], RELU, bias=gb1t)
                        pg2 = pq.tile([128, TILE_N], f32, tag="psq")
                        nc.tensor.matmul(pg2[:], BD["gw2"], g1s[:],
                                         start=True, stop=True)
                        g2s = tp.tile([128, TILE_N], f16, tag="g2s")
                        nc.scalar.activation(g2s[:], pg2[:], RELU, bias=gb2t)
                        pg3 = pq.tile([128, TILE_N], f32, tag="psq")
                        nc.tensor.matmul(pg3[:], BD["gw3r"], g2s[:],
                                         start=True, stop=True)
                        nc.scalar.activation(gate_q[:, sl], pg3[:], IDENT,
                                             bias=gb3t)

                # sentinel column for empty-slot gathers
                nc.vector.memset(gate_q[:, QW:QW + 1], SENT)

                # per-graph max; extra slot NS = +1e5 kills foreign columns
                gmax = p3.tile([128, NS + 1], f32)
                nc.vector.memset(gmax[:, NS:NS + 1], 1.0e5)
                with nc.named_scope("GMAX"), \
                        tc.tile_pool(name="zp", bufs=1) as zp:
                    for p, gs_t in ((0, gsa), (1, gsb)):
                        Zs = zp.tile([128, NSP, SW], f32, tag="Zs")
                        nc.gpsimd.ap_gather(
                            Zs[:], gate_q[:], gs_t[:],
                            channels=128, num_elems=QW + 1, d=1,
                            num_idxs=NSP * SW)
                        nc.vector.tensor_reduce(
                            gmax[:, p * NSP:(p + 1) * NSP], Zs[:],
                            mybir.AxisListType.X, mybir.AluOpType.max)

                # second pass: attn mlp, exp, prefix sums
                csE = p3.tile([128, 1 + QW], f32)
                csW = p3.tile([128, 1 + QW], f32)
                nc.vector.memset(csE[:, 0:1], 0.0)
                nc.vector.memset(csW[:, 0:1], 0.0)
                NT = QW // TILE_N  # 19
                with nc.named_scope("PATT"), \
                        tc.tile_pool(name="mp", bufs=1) as mp:
                    # one decorrelated broadcast gather: col b*NT+a holds
                    # gmax[slot(node a*512+b)]
                    Mfull = mp.tile([128, TILE_N, NT], f32)
                    nc.gpsimd.ap_gather(
                        Mfull[:], gmax[:], gnode[:],
                        channels=128, num_elems=NS + 1, d=1, num_idxs=QW)
                    for t in range(NT):
                        t0 = t * TILE_N
                        sl = slice(t0, t0 + TILE_N)
                        pt1 = pq.tile([128, TILE_N], f32, tag="psq")
                        nc.tensor.matmul(pt1[:], BD["aw1"], h2q[:, sl],
                                         start=True, stop=True)
                        t1s = tp.tile([128, TILE_N], f16, tag="g1s")
                        nc.scalar.activation(t1s[:], pt1[:], RELU, bias=ab1t)
                        pt2 = pq.tile([128, TILE_N], f32, tag="psq")
                        nc.tensor.matmul(pt2[:], BD["aw2"], t1s[:],
                                         start=True, stop=True)
                        t2s = tp.tile([128, TILE_N], f32, tag="t2s")
                        nc.scalar.activation(t2s[:], pt2[:], RELU, bias=ab2t)

                        Ep = tp.tile([128, TILE_N], f32, tag="Ep")
                        nc.vector.tensor_tensor(Ep[:], gate_q[:, sl],
                                                Mfull[:, :, t], SUB)
                        Ee = tp.tile([128, TILE_N], f32, tag="Ee")
                        nc.scalar.activation(Ee[:], Ep[:], EXP)
                        wt = tp.tile([128, TILE_N], f32, tag="wt")
                        nc.vector.tensor_tensor(wt[:], Ee[:], t2s[:], MUL)
                        nc.vector._custom_dve(
                            CUMSUM, out=csE[:, 1 + t0:1 + t0 + TILE_N],
                            in0=Ee[:], s0=csE[:, t0:t0 + 1])
                        nc.vector._custom_dve(
                            CUMSUM, out=csW[:, 1 + t0:1 + t0 + TILE_N],
                            in0=wt[:], s0=csW[:, t0:t0 + 1])

                with nc.named_scope("PFIN"):
                    GdE = p3.tile([128, 1 + NS], f32)
                    GdW = p3.tile([128, 1 + NS], f32)
                    nc.vector.memset(GdE[:, 0:1], 0.0)
                    nc.vector.memset(GdW[:, 0:1], 0.0)
                    nc.gpsimd.ap_gather(
                        GdE[:, 1:], csE[:], gend[:],
                        channels=128, num_elems=1 + QW, d=1, num_idxs=NS)
                    nc.gpsimd.ap_gather(
                        GdW[:, 1:], csW[:], gend[:],
                        channels=128, num_elems=1 + QW, d=1, num_idxs=NS)
                    denom = p3.tile([128, NS], f32)
                    num = p3.tile([128, NS], f32)
                    nc.vector.tensor_tensor(denom[:], GdE[:, 1:], GdE[:, :-1],
                                            SUB)
                    nc.vector.tensor_tensor(num[:], GdW[:, 1:], GdW[:, :-1], SUB)
                    nc.vector.tensor_scalar_max(denom[:], denom[:], 1e-16)
                    rec = p3.tile([128, NS], f32)
                    nc.vector.reciprocal(rec[:], denom[:])
                    pooled = p3.tile([128, NS], f16)
                    nc.vector.tensor_tensor(pooled[:], num[:], rec[:], MUL)

                    pc1 = pq.tile([128, NS], f32, tag="psc", bufs=2)
                    nc.tensor.matmul(pc1[:], BD["fw1"], pooled[:],
                                     start=True, stop=True)
                    c1s = p3.tile([128, NS], f16)
                    nc.scalar.activation(c1s[:], pc1[:], RELU, bias=fb1t)
                    pc2 = pq.tile([128, NS], f32, tag="psc", bufs=2)
                    nc.tensor.matmul(pc2[:], BD["fw2"], c1s[:],
                                     start=True, stop=True)
                    c2s = p3.tile([128, NS], f16)
                    nc.scalar.activation(c2s[:], pc2[:], RELU, bias=fb2t)
                    pc3 = pq.tile([128, NS], f32, tag="psc", bufs=2)
                    nc.tensor.matmul(pc3[:], BD["fw3r"], c2s[:],
                                     start=True, stop=True)
                    o3 = p3.tile([128, NS], f32)
                    nc.scalar.activation(o3[:], pc3[:], IDENT, bias=fb3t)
                    for q in range(4):
                        nc.sync.dma_start(out_g.ap()[q:q + 1, :],
                                          o3[32 * q:32 * q + 1, :])

    nc.compile()
    _split_multi_waits(nc, mybir)
    return nc


# ================================================================ entry
def kernel(x, w1, b1, w2, b2, gw1, gb1, gw2, gb2, gw3, gb3,
           aw1, ab1, aw2, ab2, fw1, fb1, fw2, fb2, fw3, fb3,
           edge_index, batch_vec, num_graphs):
    from concourse.bass_utils import run_bass_kernel_spmd

    x = np.asarray(x, np.float32)
    cores, bounds, nvi = _prep(edge_index, batch_vec)

    f32a = lambda a: np.asarray(a, np.float32)
    f16a = lambda a: np.ascontiguousarray(np.asarray(a, np.float32)
                                          .astype(np.float16))

    w1n, w2n = f32a(w1), f32a(w2)

    xt = np.zeros((128, BLK), np.float32)
    for k in range(NC):
        xt[16 * k:16 * (k + 1), :] = x[BLK * k:BLK * (k + 1), :].T

    def bd4(w):
        out = np.zeros((128, 128), np.float32)
        for q in range(4):
            out[32 * q:32 * (q + 1), 32 * q:32 * (q + 1)] = w
        return out

    gw3r = np.tile(f32a(gw3).reshape(H, 1), (1, H))
    fw3r = np.tile(f32a(fw3).reshape(H, 1), (1, H))

    wp16 = np.zeros((128, 1216), np.float32)
    wp16[:, 0:16] = np.tile(w1n[:, 0:16], (8, 1))
    wp16[:, 16:32] = np.tile(w1n[:, 16:32], (8, 1))
    wp16[0:16, 32:48] = w1n[:, 0:16]
    wp16[0:16, 48:64] = w1n[:, 16:32]
    wp16[:, 64:96] = np.tile(w2n[0:16, :], (8, 1))
    wp16[:, 96:128] = np.tile(w2n[16:32, :], (8, 1))
    wp16[0:16, 128:160] = w2n[0:16, :]
    wp16[0:16, 160:192] = w2n[16:32, :]
    for i, w in enumerate((gw1, gw2, gw3r, aw1, aw2, fw1, fw2, fw3r)):
        wp16[:, 192 + 128 * i:192 + 128 * (i + 1)] = bd4(f32a(w))
    wp16 = wp16.astype(np.float16)

    def t4(b):
        return np.tile(f32a(b).reshape(H), 4)

    wp32 = np.zeros((128, 16), np.float32)
    wp32[0:16, 0] = f32a(b1).reshape(-1)[0:16]
    wp32[0:16, 1] = f32a(b1).reshape(-1)[16:32]
    wp32[0:32, 2] = f32a(b2).reshape(-1)
    wp32[:, 3] = t4(gb1)
    wp32[:, 4] = t4(gb2)
    wp32[:, 5] = float(np.asarray(gb3).reshape(-1)[0])
    wp32[:, 6] = t4(ab1)
    wp32[:, 7] = t4(ab2)
    wp32[:, 8] = t4(fb1)
    wp32[:, 9] = t4(fb2)
    wp32[:, 10] = float(np.asarray(fb3).reshape(-1)[0])

    common = dict(xt=xt, wp16=wp16, wp32=wp32)

    in_maps = []
    for c, info in enumerate(cores):
        xo = np.zeros((16, NMAX), np.float16)
        xo[:, :info['size']] = \
            x[info['n_lo']:info['n_lo'] + info['size'], :].T.astype(np.float16)
        m = dict(common)
        m.update(xo=xo, ge=info['ge'], gd=info['gd'], gsa=info['gsa'],
                 gsb=info['gsb'], gnode=info['gnode'], gend=info['gend'])
        in_maps.append(m)

    key = (tuple(bounds), tuple(nvi))
    if _cache.get('key') != key:
        _cache['nc'] = _build_program(bounds, nvi)
        _cache['key'] = key
    ncp = _cache['nc']

    res = run_bass_kernel_spmd(ncp, in_maps, core_ids=list(range(NC)),
                               trace=bool(os.environ.get("KERNEL_TRACE")))
    _cache['last_results'] = res

    out = np.zeros((N_GRAPHS, 1), np.float32)
    for c, info in enumerate(cores):
        vals = np.asarray(res.results[c]["outg"])  # [4, NS]
        for q in range(4):
            for s in range(NS):
                g = info['slot_map'][q, s]
                if g >= 0:
                    out[info['g_lo'] + g, 0] = vals[q, s]
    return out


# revision 46
# speedup vs baseline: 1.3214x; 1.0504x over previous
"""2-layer GIN + attentional pooling on 8 Trainium2 NeuronCores (Bass/Tile).

v2 rewrite of the gather/cumsum baseline:
  - Nodes split into 8 graph-aligned ownership ranges (one per core); each
    core processes edges whose dst it owns, bucketed by 32768-node src block
    (one per GPSIMD core-group) and dst-ordered within 1024-node chunks.
  - Segment sums over dst: DVE prefix-scan over the dst-sorted edge stream,
    ap_gather of the cumsum at host-known segment ends, shifted subtraction.
  - GIN linear fused into the block fold: (x + A.x) @ w == w_blk-fold(P) + w.x,
    so each 512-node tile is 2 (L1) / 3 (L2) fp16 matmuls, single PE pass.
  - h1 exchanged with AllGather (f16, feature pairs (j, j+16) interleaved for
    the 4-byte d=2 gather granule); a plain [32, N] copy kept for the local
    self term.
  - Pooling: exact per-graph softmax. Nodes re-laid 4 graph-aligned quarters
    x 32 features across 128 partitions; gate/attn MLPs as block-diagonal
    128x128 fp16 matmuls; per-graph gate max via ap_gather into padded
    per-graph slots + 3D tensor_reduce; exp(gate - max) with the max
    broadcast back by a second tiny gather; denominator and weighted sums via
    per-quarter prefix scans probed at graph ends.
"""
import os
import sys

os.environ.setdefault("NEURON_RT_RESET_CORES", "1")
sys.path.insert(0, '/opt/trn_rl_repo')

import numpy as np


# -- NTFF profiling hook shim (optional; enables trace=True under axon) ----
def _install_ntff_shim():
    import types
    try:
        import antenv
        if 'antenv.axon_hooks' in sys.modules:
            return
        hooks = types.ModuleType('antenv.axon_hooks')
        _state = {'hook': None}
        hooks.set_axon_ntff_profile_hook = lambda h: _state.__setitem__('hook', h)
        hooks.get_axon_ntff_profile_hook = lambda: _state['hook']
        sys.modules['antenv.axon_hooks'] = hooks
        antenv.axon_hooks = hooks
        from trn_agent_boot.trn_boot import _ntff_profile_via_ctypes
        h = _ntff_profile_via_ctypes('/opt/axon/libaxon_pjrt.so')
        if h is not None:
            hooks.set_axon_ntff_profile_hook(h)
    except Exception:
        pass


_install_ntff_shim()

N_NODES = 262144
N_GRAPHS = 1024
C_IN = 16
H = 32
NC = 8
BLK = 32768
NCH = 1024                 # dst nodes per chunk
NCHUNK = 33
ECH = 2368                 # edge capacity per (block, chunk); max seen 2220
NMAX = NCH * NCHUNK        # 33792
QSTEP = NMAX // 4          # 8448: fixed quarter stride (same on all cores)
QW = 9728                  # 19*512: fixed quarter window width
QOFF = (0, QSTEP - 512, 2 * QSTEP - 512, 3 * QSTEP - 512)  # static offsets
H2PAD = QOFF[3] + QW - NMAX  # 768 zero-padded columns after h2
NSP = 24                   # graph slots per gmax gather pass
NS = 2 * NSP               # graph slots per quarter window (max seen 34)
SW = 320                   # slot width >= max graph size (max seen 317)
SENT = -1.0e5
TILE_N = 512
MAX_WAITS = 1

_cache = {}


def _split_multi_waits(nc, mybir, max_waits=MAX_WAITS):
    n_split = 0
    for fn in nc.m.functions:
        for bb in fn.blocks:
            out = []
            for ins in bb.instructions:
                si = ins.sync_info
                if si is not None and si.on_wait and len(si.on_wait) > max_waits:
                    waits = list(si.on_wait)
                    extra = waits[:-max_waits]
                    keep = waits[-max_waits:]
                    for i in range(0, len(extra), max_waits):
                        group = extra[i:i + max_waits]
                        nop = mybir.InstNoOp(
                            name=f"waitsplit_{nc.next_id()}",
                            sync_info=mybir.SyncInfo(on_wait=group, on_update=[]),
                            bass_nofuse=True,
                            engine=ins.engine,
                        )
                        out.append(nop)
                        n_split += 1
                    si.on_wait = keep
                out.append(ins)
            bb.instructions = out
    return n_split


def _wrap_idx(vals, group, arr, col0=0):
    """Wrapped ap_gather index layout: value i -> arr[16g + i%16, col0 + i//16]."""
    n = len(vals)
    assert n % 16 == 0
    v = np.asarray(vals, dtype=np.int16).reshape(n // 16, 16).T
    arr[16 * group:16 * group + 16, col0:col0 + n // 16] = v


def _register_cumsum():
    from concourse import dve_ops
    from concourse.dve_spec import Spec, Src0, C0, AluOp, lower
    import concourse.dve_spec as ds
    from concourse.dve_uop import DveOpSpec
    for op in dve_ops.OPS:
        if op.name == "CUMSUM_ANT":
            return op
    spec = Spec(
        body=ds.scan(AluOp.ADD, Src0, init=C0),
        reference=lambda in0, s0: np.cumsum(in0.astype(np.float32), axis=-1) + s0,
    )
    shas = {}
    for ver in ("v3", "v4"):
        uops = lower(spec, ver=ver)
        shas[ver] = DveOpSpec(name="CUMSUM_ANT", opcode=1, uops=uops,
                              rd1_en=False).sha(ver)
    op = dve_ops.DveOp("CUMSUM_ANT", spec, subdim=False, uops_sha=shas)
    dve_ops.OPS.append(op)
    dve_ops.CUSTOM_DVE_SPECS["CUMSUM_ANT"] = spec
    dve_ops._SUB_OPCODE_FOR_NAME["CUMSUM_ANT"] = \
        max(dve_ops._SUB_OPCODE_FOR_NAME.values()) + 1
    return op


# ================================================================ host prep
def _prep(edge_index, batch_vec):
    src = np.asarray(edge_index[0], dtype=np.int64)
    dst = np.asarray(edge_index[1], dtype=np.int64)
    bv = np.asarray(batch_vec, dtype=np.int64)

    gstart = np.searchsorted(bv, np.arange(N_GRAPHS))
    bounds = [0]
    for c in range(1, NC):
        target = c * (N_NODES // NC)
        gi = np.searchsorted(gstart, target)
        cand = []
        if gi < N_GRAPHS:
            cand.append(int(gstart[gi]))
        if gi > 0:
            cand.append(int(gstart[gi - 1]))
        bounds.append(min(cand, key=lambda v: abs(v - target)))
    bounds.append(N_NODES)
    n_lo = np.array(bounds[:-1])
    n_hi = np.array(bounds[1:])
    sizes = n_hi - n_lo
    assert sizes.max() <= NMAX, sizes
    g_lo = np.searchsorted(gstart, n_lo)
    g_hi = np.searchsorted(gstart, n_hi)

    owner = np.searchsorted(n_hi, dst, side='right')

    nvi = np.zeros(NCHUNK, np.int64)  # per-chunk edge gather length (all cores)
    cores = []
    for c in range(NC):
        m = owner == c
        csrc = src[m]
        cdst_local = dst[m] - n_lo[c]
        size_c = int(sizes[c])

        ge = np.zeros((128, NCHUNK * ECH // 16), np.int16)
        gd = np.zeros((128, NCHUNK * NCH // 16), np.int16)

        blk_of = csrc >> 15
        src_local_all = (csrc & (BLK - 1))

        for k in range(NC):
            bm = blk_of == k
            bsrc = src_local_all[bm]
            bdst = cdst_local[bm]
            order = np.argsort(bdst, kind='stable')
            bsrc = bsrc[order].astype(np.int16)
            bdst = bdst[order]
            cnt = np.bincount(bdst, minlength=NMAX)
            cum = np.concatenate([[0], np.cumsum(cnt)])

            for ch in range(NCHUNK):
                a, b = ch * NCH, (ch + 1) * NCH
                e0, e1 = cum[a], cum[b]
                ne = int(e1 - e0)
                assert ne <= ECH, (c, k, ch, ne, ECH)
                nvi[ch] = max(nvi[ch], -(-ne // 16) * 16)
                ev = np.zeros(ECH, np.int16)
                ev[:ne] = bsrc[e0:e1]
                _wrap_idx(ev, k, ge, col0=ch * ECH // 16)
                ends = (cum[a + 1:b + 1] - e0).astype(np.int16)
                # transposed probe stream: slot s=b64*64+a64 -> node a64*16+b64
                # (decorrelates consecutive gather addresses)
                stream = ends.reshape(64, 16).T.flatten()
                _wrap_idx(stream, k, gd, col0=ch * NCH // 16)

        # ---- pooling quarter windows (fixed offsets, per-core content) ----
        glo, ghi = int(g_lo[c]), int(g_hi[c])
        ls = (gstart[glo:ghi] - n_lo[c]).astype(np.int64)       # graph starts
        le = np.concatenate([ls[1:], [size_c]]).astype(np.int64)  # graph ends
        q_of = ls // QSTEP  # quarter window owning each graph (by start node)

        gsa = np.zeros((128, NSP * SW // 16), np.int16)
        gnode = np.zeros((128, QW // 2 // 16), np.int16)
        gend = np.zeros((128, NS // 16), np.int16)
        slot_map = np.full((4, NS), -1, np.int64)
        for q in range(4):
            off = QOFF[q]
            sel = np.where(q_of == q)[0]
            ngq = len(sel)
            assert ngq <= NS, (c, q, ngq)
            starts = ls[sel] - off
            ends = le[sel] - off
            lens = ends - starts
            if ngq:
                assert starts.min() >= 0 and ends.max() <= QW, (c, q)
                assert lens.max() <= SW, (c, q, lens.max())
            for s in range(ngq):
                slot_map[q, s] = int(sel[s])  # graph idx local to core

            # slot gather streams: the quarter's two GPSIMD groups each cover
            # half the slots (one gather pass); pads cycle through the slot's
            # own columns
            w_ar = np.arange(SW)
            for p, g2 in ((0, 2 * q), (1, 2 * q + 1)):
                ev = np.empty(NSP * SW, np.int16)
                for si in range(NSP):
                    s = p * NSP + si
                    if s < ngq:
                        ev[si * SW:(si + 1) * SW] = \
                            (starts[s] + w_ar % lens[s]).astype(np.int16)
                    else:
                        ev[si * SW:(si + 1) * SW] = w_ar.astype(np.int16)
                _wrap_idx(ev, g2, gsa)

            # node -> slot stream; foreign/pad columns -> slot NS (+1e5 max).
            # Each group gathers one half of the quarter's nodes.
            evn = np.full(QW, NS, np.int16)
            for s in range(ngq):
                evn[starts[s]:ends[s]] = s
            _wrap_idx(evn[:QW // 2], 2 * q, gnode)
            _wrap_idx(evn[QW // 2:], 2 * q + 1, gnode)

            # slot -> cumsum end-probe offset stream
            eve = np.zeros(NS, np.int16)
            prev = int(starts[0]) if ngq else 0
            for s in range(NS):
                if s < ngq:
                    prev = int(ends[s])
                eve[s] = prev
            _wrap_idx(eve, 2 * q, gend)
            _wrap_idx(eve, 2 * q + 1, gend)

        cores.append(dict(
            n_lo=int(n_lo[c]), size=size_c, g_lo=glo, g_hi=ghi,
            ge=ge, gd=gd, gsa=gsa, gnode=gnode, gend=gend,
            slot_map=slot_map,
        ))
    return cores, [int(b) for b in bounds], [int(v) for v in nvi]


# ================================================================ device
def _build_program(bounds, nvi):
    from concourse import bacc, tile
    from concourse.bass import mybir

    CUMSUM = _register_cumsum()

    f32 = mybir.dt.float32
    f16 = mybir.dt.float16
    i16 = mybir.dt.int16
    RELU = mybir.ActivationFunctionType.Relu
    EXP = mybir.ActivationFunctionType.Exp
    IDENT = mybir.ActivationFunctionType.Identity
    SUB = mybir.AluOpType.subtract
    MUL = mybir.AluOpType.mult

    nc = bacc.Bacc("TRN2", target_bir_lowering=False, debug=False, num_devices=NC)

    def din(name, shape, dt):
        return nc.dram_tensor(name, shape, dt, kind="ExternalInput")

    xt_in = din("xt", [128, BLK], f32)
    xo_in = din("xo", [16, NMAX], f16)
    ge_in = din("ge", [128, NCHUNK * ECH // 16], i16)
    gd_in = din("gd", [128, NCHUNK * NCH // 16], i16)
    gsa_in = din("gsa", [128, NSP * SW // 16], i16)
    gnode_in = din("gnode", [128, QW // 2 // 16], i16)
    gend_in = din("gend", [128, NS // 16], i16)
    wp16_in = din("wp16", [128, 1216], f16)
    wp32_in = din("wp32", [128, 16], f32)

    out_g = nc.dram_tensor("outg", [4, NS], f32, kind="ExternalOutput")

    h1i_own = nc.dram_tensor("h1i_own", [16, NMAX, 2], f16)
    h1i_all = nc.dram_tensor("h1i_all", [NC * 16, NMAX, 2], f16,
                             addr_space="Shared")
    h2_dram = nc.dram_tensor("h2d", [32, NMAX + H2PAD], f16)

    with tile.TileContext(nc) as tc:
        with (
            tc.tile_pool(name="cw", bufs=1) as cw,
        ):
            wp16 = cw.tile([128, 1216], f16, name="wp16")
            nc.sync.dma_start(wp16[:], wp16_in.ap()[:])
            wp32 = cw.tile([128, 16], f32, name="wp32")
            nc.sync.dma_start(wp32[:], wp32_in.ap()[:])

            zpad = cw.tile([32, H2PAD], f16, name="zpad")
            nc.vector.memset(zpad[:], 0.0)
            nc.sync.dma_start(h2_dram.ap()[:, NMAX:NMAX + H2PAD], zpad[:])

            w1blk_a = wp16[:, 0:16]
            w1blk_b = wp16[:, 16:32]
            w1s_a = wp16[0:16, 32:48]
            w1s_b = wp16[0:16, 48:64]
            w2blk0 = wp16[:, 64:96]
            w2blk1 = wp16[:, 96:128]
            w2s_a = wp16[0:16, 128:160]
            w2s_b = wp16[0:16, 160:192]
            BD = {}
            for i, nm in enumerate(("gw1", "gw2", "gw3r", "aw1", "aw2",
                                    "fw1", "fw2", "fw3r")):
                BD[nm] = wp16[:, 192 + 128 * i:192 + 128 * (i + 1)]
            b1a = wp32[0:16, 0:1]
            b1b = wp32[0:16, 1:2]
            b2f = wp32[0:32, 2:3]
            gb1t = wp32[:, 3:4]
            gb2t = wp32[:, 4:5]
            gb3t = wp32[:, 5:6]
            ab1t = wp32[:, 6:7]
            ab2t = wp32[:, 7:8]
            fb1t = wp32[:, 8:9]
            fb2t = wp32[:, 9:10]
            fb3t = wp32[:, 10:11]

            with (
                tc.tile_pool(name="tbl", bufs=1) as tblp,
            ):
                # ---------------- Layer 1 ----------------
                table1 = tblp.tile([128, BLK], f32, tag="table")
                nc.sync.dma_start(table1[:], xt_in.ap()[:])

                with nc.named_scope("L1"), (
                    tc.tile_pool(name="ip", bufs=4)) as ip, (
                    tc.tile_pool(name="xp", bufs=3)) as xp, (
                    tc.tile_pool(name="cp", bufs=2)) as cp, (
                    tc.tile_pool(name="gp", bufs=2)) as gp, (
                    tc.tile_pool(name="op", bufs=3)) as op, (
                    tc.tile_pool(name="pp", bufs=3, space="PSUM")) as pp:
                    for ch in range(NCHUNK):
                        n = nvi[ch]
                        gidx = ip.tile([128, ECH // 16], i16, tag="gidx")
                        nc.sync.dma_start(
                            gidx[:, :n // 16],
                            ge_in.ap()[:, ch * ECH // 16:
                                       ch * ECH // 16 + n // 16])
                        didx = ip.tile([128, NCH // 16], i16, tag="didx")
                        nc.sync.dma_start(
                            didx[:],
                            gd_in.ap()[:, ch * NCH // 16:(ch + 1) * NCH // 16])
                        xoc = xp.tile([16, NCH], f16, tag="xoc")
                        nc.sync.dma_start(
                            xoc[:], xo_in.ap()[:, ch * NCH:(ch + 1) * NCH])

                        cs = cp.tile([128, 1 + ECH], f32, tag="cs")
                        nc.vector.memset(cs[:, 0:1], 0.0)
                        nc.gpsimd.ap_gather(
                            cs[:, 1:1 + n], table1[:], gidx[:, :n // 16],
                            channels=128, num_elems=BLK, d=1, num_idxs=n)
                        nc.vector._custom_dve(
                            CUMSUM, out=cs[:, 1:1 + n], in0=cs[:, 1:1 + n],
                            s0=0.0)

                        # probes in transposed stream order: slot (b,a) holds
                        # cs[end of node a*16+b]
                        Gp = gp.tile([128, 16, 64], f32, tag="G")
                        nc.gpsimd.ap_gather(
                            Gp[:], cs[:], didx[:],
                            channels=128, num_elems=1 + ECH, d=1, num_idxs=NCH)
                        Pp = gp.tile([128, 16, 64], f16, tag="P")
                        nc.vector.tensor_tensor(Pp[:, 1:16, :], Gp[:, 1:16, :],
                                                Gp[:, 0:15, :], SUB)
                        nc.vector.tensor_tensor(Pp[:, 0, 1:64], Gp[:, 0, 1:64],
                                                Gp[:, 15, 0:63], SUB)
                        nc.vector.tensor_tensor(Pp[:, 0, 0:1], Gp[:, 0, 0:1],
                                                cs[:, 0:1], SUB)
                        PpT = Pp[:].transpose([0, 2, 1])  # [128, 64(a), 16(b)]

                        for t0 in range(0, NCH, TILE_N):
                            tn = TILE_N
                            a0 = t0 // 16
                            sl = slice(t0, t0 + tn)
                            rhsP = PpT[:, a0:a0 + 32, :]
                            pha = pp.tile([16, tn], f32, tag="psa")
                            nc.tensor.matmul(pha[:], w1blk_a, rhsP,
                                             start=True, stop=False)
                            nc.tensor.matmul(pha[:], w1s_a, xoc[:, sl],
                                             start=False, stop=True)
                            phb = pp.tile([16, tn], f32, tag="psb")
                            nc.tensor.matmul(phb[:], w1blk_b, rhsP,
                                             start=True, stop=False)
                            nc.tensor.matmul(phb[:], w1s_b, xoc[:, sl],
                                             start=False, stop=True)
                            he3 = op.tile([16, tn, 2], f16, tag="he3")
                            nc.scalar.activation(he3[:, :, 0], pha[:], RELU,
                                                 bias=b1a)
                            nc.scalar.activation(he3[:, :, 1], phb[:], RELU,
                                                 bias=b1b)
                            col = ch * NCH + t0
                            nc.sync.dma_start(
                                h1i_own.ap()[:, col:col + tn, :], he3[:])

                # ---------------- exchange ----------------
                with nc.named_scope("AG"):
                    nc.gpsimd.collective_compute(
                        "AllGather", mybir.AluOpType.bypass,
                        replica_groups=[list(range(NC))],
                        ins=[h1i_own.ap()[:]],
                        outs=[h1i_all.ap()[:]],
                    )

                # ---------------- table2 ----------------
                table2 = tblp.tile([128, BLK, 2], f16, tag="table")
                with nc.named_scope("T2"):
                    for k in range(NC):
                        lo, hi = k * BLK, (k + 1) * BLK
                        pos = lo
                        while pos < hi:
                            c2 = next(i for i in range(NC)
                                      if bounds[i] <= pos < bounds[i + 1])
                            seg_end = min(hi, bounds[c2 + 1])
                            ln = seg_end - pos
                            local = pos - bounds[c2]
                            nc.sync.dma_start(
                                table2[16 * k:16 * (k + 1),
                                       pos - lo:pos - lo + ln, :],
                                h1i_all.ap()[16 * c2:16 * (c2 + 1),
                                             local:local + ln, :])
                            pos = seg_end

                # ---------------- Layer 2 ----------------
                with nc.named_scope("L2"), (
                    tc.tile_pool(name="ip2", bufs=4)) as ip, (
                    tc.tile_pool(name="xp2", bufs=3)) as xp, (
                    tc.tile_pool(name="cp2", bufs=2)) as cp, (
                    tc.tile_pool(name="gp2", bufs=2)) as gp, (
                    tc.tile_pool(name="op2", bufs=3)) as op, (
                    tc.tile_pool(name="pp2", bufs=4, space="PSUM")) as pp:
                    for ch in range(NCHUNK):
                        n = nvi[ch]
                        gidx = ip.tile([128, ECH // 16], i16, tag="gidx")
                        nc.sync.dma_start(
                            gidx[:, :n // 16],
                            ge_in.ap()[:, ch * ECH // 16:
                                       ch * ECH // 16 + n // 16])
                        didx = ip.tile([128, NCH // 16], i16, tag="didx")
                        nc.sync.dma_start(
                            didx[:],
                            gd_in.ap()[:, ch * NCH // 16:(ch + 1) * NCH // 16])
                        h1c = xp.tile([16, NCH, 2], f16, tag="h1c")
                        nc.sync.dma_start(
                            h1c[:], h1i_own.ap()[:, ch * NCH:(ch + 1) * NCH, :])

                        stage = cp.tile([128, ECH, 2], f16, tag="stage")
                        nc.gpsimd.ap_gather(
                            stage[:, :n, :], table2[:], gidx[:, :n // 16],
                            channels=128, num_elems=BLK, d=2, num_idxs=n)
                        cs2 = cp.tile([128, 1 + ECH, 2], f32, tag="cs2", bufs=1)
                        nc.vector.memset(cs2[:, 0:1, :], 0.0)
                        nc.vector._custom_dve(
                            CUMSUM, out=cs2[:, 1:1 + n, 0], in0=stage[:, :n, 0],
                            s0=0.0)
                        nc.vector._custom_dve(
                            CUMSUM, out=cs2[:, 1:1 + n, 1], in0=stage[:, :n, 1],
                            s0=0.0)

                        G2 = gp.tile([128, 16, 64, 2], f32, tag="G2", bufs=1)
                        nc.gpsimd.ap_gather(
                            G2[:], cs2[:], didx[:],
                            channels=128, num_elems=1 + ECH, d=2, num_idxs=NCH)
                        P2 = gp.tile([128, 16, 64, 2], f16, tag="P2")
                        nc.vector.tensor_tensor(P2[:, 1:16, :, :],
                                                G2[:, 1:16, :, :],
                                                G2[:, 0:15, :, :], SUB)
                        nc.vector.tensor_tensor(P2[:, 0, 1:64, :],
                                                G2[:, 0, 1:64, :],
                                                G2[:, 15, 0:63, :], SUB)
                        nc.vector.tensor_tensor(P2[:, 0, 0:1, :],
                                                G2[:, 0, 0:1, :],
                                                cs2[:, 0:1, :], SUB)
                        P2T = P2[:].transpose([0, 2, 1, 3])  # [128,64,16,2]

                        for t0 in range(0, NCH, TILE_N):
                            tn = TILE_N
                            a0 = t0 // 16
                            sl = slice(t0, t0 + tn)
                            ph2 = pp.tile([H, tn], f32, tag="ps2")
                            nc.tensor.matmul(ph2[:], w2blk0,
                                             P2T[:, a0:a0 + 32, :, 0],
                                             start=True, stop=False)
                            nc.tensor.matmul(ph2[:], w2blk1,
                                             P2T[:, a0:a0 + 32, :, 1],
                                             start=False, stop=False)
                            nc.tensor.matmul(ph2[:], w2s_a, h1c[:, sl, 0],
                                             start=False, stop=False)
                            nc.tensor.matmul(ph2[:], w2s_b, h1c[:, sl, 1],
                                             start=False, stop=True)
                            h2t = op.tile([32, tn], f16, tag="h2t")
                            nc.scalar.activation(h2t[:], ph2[:], RELU, bias=b2f)
                            col = ch * NCH + t0
                            nc.sync.dma_start(
                                h2_dram.ap()[:, col:col + tn], h2t[:])

            # ---------------- pooling ----------------
            with (
                tc.tile_pool(name="p3", bufs=1) as p3,
                tc.tile_pool(name="tp", bufs=2) as tp,
                tc.tile_pool(name="pq", bufs=4, space="PSUM") as pq,
            ):
                gsa = p3.tile([128, NSP * SW // 16], i16)
                nc.sync.dma_start(gsa[:], gsa_in.ap()[:])
                gnode = p3.tile([128, QW // 2 // 16], i16)
                nc.sync.dma_start(gnode[:], gnode_in.ap()[:])
                gend = p3.tile([128, NS // 16], i16)
                nc.sync.dma_start(gend[:], gend_in.ap()[:])

                h2q = p3.tile([128, QW], f16)
                for q in range(4):
                    nc.sync.dma_start(
                        h2q[32 * q:32 * (q + 1), :],
                        h2_dram.ap()[:, QOFF[q]:QOFF[q] + QW])

                gate_q = p3.tile([128, QW + 1], f32)
                with nc.named_scope("PMLP"):
                    for t0 in range(0, QW, TILE_N):
                        sl = slice(t0, t0 + TILE_N)
                        pg1 = pq.tile([128, TILE_N], f32, tag="psq")
                        nc.tensor.matmul(pg1[:], BD["gw1"], h2q[:, sl],
                                         start=True, stop=True)
                        g1s = tp.tile([128, TILE_N], f16, tag="g1s")
                        nc.scalar.activation(g1s[:], pg1[:], RELU, bias=gb1t)
                        pg2 = pq.tile([128, TILE_N], f32, tag="psq")
                        nc.tensor.matmul(pg2[:], BD["gw2"], g1s[:],
                                         start=True, stop=True)
                        g2s = tp.tile([128, TILE_N], f16, tag="g2s")
                        nc.scalar.activation(g2s[:], pg2[:], RELU, bias=gb2t)
                        pg3 = pq.tile([128, TILE_N], f32, tag="psq")
                        nc.tensor.matmul(pg3[:], BD["gw3r"], g2s[:],
                                         start=True, stop=True)
                        nc.scalar.activation(gate_q[:, sl], pg3[:], IDENT,
                                             bias=gb3t)

                # sentinel column for empty-slot gathers
                nc.vector.memset(gate_q[:, QW:QW + 1], SENT)

                # per-graph max; extra slot NS = +1e5 kills foreign columns.
                # One gather pass: each quarter's two groups cover half the
                # slots, then tiny DMAs replicate both halves to all rows.
                gmax = p3.tile([128, NS + 1], f32)
                nc.vector.memset(gmax[:, NS:NS + 1], 1.0e5)
                with nc.named_scope("GMAX"), \
                        tc.tile_pool(name="zp", bufs=1) as zp:
                    Zs = zp.tile([128, NSP, SW], f32, tag="Zs")
                    nc.gpsimd.ap_gather(
                        Zs[:], gate_q[:], gsa[:],
                        channels=128, num_elems=QW + 1, d=1,
                        num_idxs=NSP * SW)
                    gmaxh = p3.tile([128, NSP], f32)
                    nc.vector.tensor_reduce(
                        gmaxh[:], Zs[:],
                        mybir.AxisListType.X, mybir.AluOpType.max)
                    for q in range(4):
                        lo, mid, hi = 32 * q, 32 * q + 16, 32 * q + 32
                        nc.sync.dma_start(gmax[lo:mid, 0:NSP],
                                          gmaxh[lo:mid, :])
                        nc.sync.dma_start(gmax[lo:mid, NSP:NS],
                                          gmaxh[mid:hi, :])
                        nc.sync.dma_start(gmax[mid:hi, 0:NSP],
                                          gmaxh[lo:mid, :])
                        nc.sync.dma_start(gmax[mid:hi, NSP:NS],
                                          gmaxh[mid:hi, :])

                # second pass: attn mlp, exp, prefix sums
                csE = p3.tile([128, 1 + QW], f32)
                csW = p3.tile([128, 1 + QW], f32)
                nc.vector.memset(csE[:, 0:1], 0.0)
                nc.vector.memset(csW[:, 0:1], 0.0)
                NT = QW // TILE_N  # 19
                HQ = QW // 2
                with nc.named_scope("PATT"), \
                        tc.tile_pool(name="mp", bufs=1) as mp:
                    # broadcast gather, halved: each quarter group gathers M
                    # for half the quarter's nodes; 3 SBUF DMAs per quarter
                    # replicate both halves onto all 32 rows.
                    M2 = mp.tile([128, QW], f32)
                    nc.gpsimd.ap_gather(
                        M2[:, 0:HQ], gmax[:], gnode[:],
                        channels=128, num_elems=NS + 1, d=1, num_idxs=HQ)
                    for q in range(4):
                        lo, mid, hi = 32 * q, 32 * q + 16, 32 * q + 32
                        nc.sync.dma_start(M2[mid:hi, HQ:QW], M2[mid:hi, 0:HQ])
                        nc.sync.dma_start(M2[lo:mid, HQ:QW], M2[mid:hi, HQ:QW])
                        nc.sync.dma_start(M2[mid:hi, 0:HQ], M2[lo:mid, 0:HQ])
                    for t in range(NT):
                        t0 = t * TILE_N
                        sl = slice(t0, t0 + TILE_N)
                        pt1 = pq.tile([128, TILE_N], f32, tag="psq")
                        nc.tensor.matmul(pt1[:], BD["aw1"], h2q[:, sl],
                                         start=True, stop=True)
                        t1s = tp.tile([128, TILE_N], f16, tag="g1s")
                        nc.scalar.activation(t1s[:], pt1[:], RELU, bias=ab1t)
                        pt2 = pq.tile([128, TILE_N], f32, tag="psq")
                        nc.tensor.matmul(pt2[:], BD["aw2"], t1s[:],
                                         start=True, stop=True)
                        t2s = tp.tile([128, TILE_N], f32, tag="t2s")
                        nc.scalar.activation(t2s[:], pt2[:], RELU, bias=ab2t)

                        Ep = tp.tile([128, TILE_N], f32, tag="Ep")
                        nc.vector.tensor_tensor(Ep[:], gate_q[:, sl],
                                                M2[:, sl], SUB)
                        Ee = tp.tile([128, TILE_N], f32, tag="Ee")
                        nc.scalar.activation(Ee[:], Ep[:], EXP)
                        wt = tp.tile([128, TILE_N], f32, tag="wt")
                        nc.vector.tensor_tensor(wt[:], Ee[:], t2s[:], MUL)
                        nc.vector._custom_dve(
                            CUMSUM, out=csE[:, 1 + t0:1 + t0 + TILE_N],
                            in0=Ee[:], s0=csE[:, t0:t0 + 1])
                        nc.vector._custom_dve(
                            CUMSUM, out=csW[:, 1 + t0:1 + t0 + TILE_N],
                            in0=wt[:], s0=csW[:, t0:t0 + 1])

                with nc.named_scope("PFIN"):
                    GdE = p3.tile([128, 1 + NS], f32)
                    GdW = p3.tile([128, 1 + NS], f32)
                    nc.vector.memset(GdE[:, 0:1], 0.0)
                    nc.vector.memset(GdW[:, 0:1], 0.0)
                    nc.gpsimd.ap_gather(
                        GdE[:, 1:], csE[:], gend[:],
                        channels=128, num_elems=1 + QW, d=1, num_idxs=NS)
                    nc.gpsimd.ap_gather(
                        GdW[:, 1:], csW[:], gend[:],
                        channels=128, num_elems=1 + QW, d=1, num_idxs=NS)
                    denom = p3.tile([128, NS], f32)
                    num = p3.tile([128, NS], f32)
                    nc.vector.tensor_tensor(denom[:], GdE[:, 1:], GdE[:, :-1],
                                            SUB)
                    nc.vector.tensor_tensor(num[:], GdW[:, 1:], GdW[:, :-1], SUB)
                    nc.vector.tensor_scalar_max(denom[:], denom[:], 1e-16)
                    rec = p3.tile([128, NS], f32)
                    nc.vector.reciprocal(rec[:], denom[:])
                    pooled = p3.tile([128, NS], f16)
                    nc.vector.tensor_tensor(pooled[:], num[:], rec[:], MUL)

                    pc1 = pq.tile([128, NS], f32, tag="psc", bufs=2)
                    nc.tensor.matmul(pc1[:], BD["fw1"], pooled[:],
                                     start=True, stop=True)
                    c1s = p3.tile([128, NS], f16)
                    nc.scalar.activation(c1s[:], pc1[:], RELU, bias=fb1t)
                    pc2 = pq.tile([128, NS], f32, tag="psc", bufs=2)
                    nc.tensor.matmul(pc2[:], BD["fw2"], c1s[:],
                                     start=True, stop=True)
                    c2s = p3.tile([128, NS], f16)
                    nc.scalar.activation(c2s[:], pc2[:], RELU, bias=fb2t)
                    pc3 = pq.tile([128, NS], f32, tag="psc", bufs=2)
                    nc.tensor.matmul(pc3[:], BD["fw3r"], c2s[:],
                                     start=True, stop=True)
                    o3 = p3.tile([128, NS], f32)
                    nc.scalar.activation(o3[:], pc3[:], IDENT, bias=fb3t)
                    for q in range(4):
                        nc.sync.dma_start(out_g.ap()[q:q + 1, :],
                                          o3[32 * q:32 * q + 1, :])

    nc.compile()
    _split_multi_waits(nc, mybir)
    return nc


# ================================================================ entry
def kernel(x, w1, b1, w2, b2, gw1, gb1, gw2, gb2, gw3, gb3,
           aw1, ab1, aw2, ab2, fw1, fb1, fw2, fb2, fw3, fb3,
           edge_index, batch_vec, num_graphs):
    from concourse.bass_utils import run_bass_kernel_spmd

    x = np.asarray(x, np.float32)
    cores, bounds, nvi = _prep(edge_index, batch_vec)

    f32a = lambda a: np.asarray(a, np.float32)
    f16a = lambda a: np.ascontiguousarray(np.asarray(a, np.float32)
                                          .astype(np.float16))

    w1n, w2n = f32a(w1), f32a(w2)

    xt = np.zeros((128, BLK), np.float32)
    for k in range(NC):
        xt[16 * k:16 * (k + 1), :] = x[BLK * k:BLK * (k + 1), :].T

    def bd4(w):
        out = np.zeros((128, 128), np.float32)
        for q in range(4):
            out[32 * q:32 * (q + 1), 32 * q:32 * (q + 1)] = w
        return out

    gw3r = np.tile(f32a(gw3).reshape(H, 1), (1, H))
    fw3r = np.tile(f32a(fw3).reshape(H, 1), (1, H))

    wp16 = np.zeros((128, 1216), np.float32)
    wp16[:, 0:16] = np.tile(w1n[:, 0:16], (8, 1))
    wp16[:, 16:32] = np.tile(w1n[:, 16:32], (8, 1))
    wp16[0:16, 32:48] = w1n[:, 0:16]
    wp16[0:16, 48:64] = w1n[:, 16:32]
    wp16[:, 64:96] = np.tile(w2n[0:16, :], (8, 1))
    wp16[:, 96:128] = np.tile(w2n[16:32, :], (8, 1))
    wp16[0:16, 128:160] = w2n[0:16, :]
    wp16[0:16, 160:192] = w2n[16:32, :]
    for i, w in enumerate((gw1, gw2, gw3r, aw1, aw2, fw1, fw2, fw3r)):
        wp16[:, 192 + 128 * i:192 + 128 * (i + 1)] = bd4(f32a(w))
    wp16 = wp16.astype(np.float16)

    def t4(b):
        return np.tile(f32a(b).reshape(H), 4)

    wp32 = np.zeros((128, 16), np.float32)
    wp32[0:16, 0] = f32a(b1).reshape(-1)[0:16]
    wp32[0:16, 1] = f32a(b1).reshape(-1)[16:32]
    wp32[0:32, 2] = f32a(b2).reshape(-1)
    wp32[:, 3] = t4(gb1)
    wp32[:, 4] = t4(gb2)
    wp32[:, 5] = float(np.asarray(gb3).reshape(-1)[0])
    wp32[:, 6] = t4(ab1)
    wp32[:, 7] = t4(ab2)
    wp32[:, 8] = t4(fb1)
    wp32[:, 9] = t4(fb2)
    wp32[:, 10] = float(np.asarray(fb3).reshape(-1)[0])

    common = dict(xt=xt, wp16=wp16, wp32=wp32)

    in_maps = []
    for c, info in enumerate(cores):
        xo = np.zeros((16, NMAX), np.float16)
        xo[:, :info['size']] = \
            x[info['n_lo']:info['n_lo'] + info['size'], :].T.astype(np.float16)
        m = dict(common)
        m.update(xo=xo, ge=info['ge'], gd=info['gd'], gsa=info['gsa'],
                 gnode=info['gnode'], gend=info['gend'])
        in_maps.append(m)

    key = (tuple(bounds), tuple(nvi))
    if _cache.get('key') != key:
        _cache['nc'] = _build_program(bounds, nvi)
        _cache['key'] = key
    ncp = _cache['nc']

    res = run_bass_kernel_spmd(ncp, in_maps, core_ids=list(range(NC)),
                               trace=bool(os.environ.get("KERNEL_TRACE")))
    _cache['last_results'] = res

    out = np.zeros((N_GRAPHS, 1), np.float32)
    for c, info in enumerate(cores):
        vals = np.asarray(res.results[c]["outg"])  # [4, NS]
        for q in range(4):
            for s in range(NS):
                g = info['slot_map'][q, s]
                if g >= 0:
                    out[info['g_lo'] + g, 0] = vals[q, s]
    return out
